# revision 12
# baseline (speedup 1.0000x reference)
"""GCMC layer on trn2 — v2: ap_gather (GPSIMD free-dim gather) + one-hot PE
segment-sum in transformed (16-lane) message space.

Design (per device, dst-sharded: device c owns dst nodes [c*NSH, (c+1)*NSH)):
  - Transform-first: x_r = (feat*cj) @ W_r  ([N,16] per rating) so each edge
    only moves 16 lanes. Table per (side, g): SBUF [128, 32000] f32 where
    partition 16k+j = lane j of slab (8g+k) (slab=6400 srcs), elem r*6400+s.
  - Edges binned per (d, g, unit k=src slab, dst-tile t, rating r); each
    (t, r) run padded to a 128-multiple of the max count over (device, unit)
    so the SPMD program is uniform. Unit streams are position-aligned: at any
    chunk all 8 units are in the same (t, r) run.
  - nc.gpsimd.ap_gather pulls per-edge x_r lanes from the SBUF table
    (per-unit int16 idx lists; Pool engine, ~6-9ns/idx/unit, 8 units in
    parallel — replaces dma_gather's ~8.6ns/row serial descriptor gen).
  - Per 128-row position: PE transpose -> T [128 e, 128 (k,j)]; DVE builds 8
    one-hots from dloc codes; 8 matmuls accumulate psum_y[dst,16] per run;
    DVE flushes into yacc [128, TP, 80] bf16.
  - Table build: stage fsrc16=(feat*cj) bf16 in DRAM; per 128-src tile PE
    transpose -> FT [64, cols]; xT = Wall^T @ FT on PE; DVE copy psum->xts
    f32; 5 HWDGE DMAs partition-remap xts -> table unit block. (NOT
    dma_start_transpose: 128B rows degenerate to per-element packets.)
  - Transform per (d, tile): relu(msg*ci) on ACT, PE transpose, f32 fc matmul
    + bias, dense store.
Host only bins/sorts indices and packs int16 idx + f32 dloc blocks.
"""
import sys
import numpy as np

sys.path.insert(0, '/opt/trn_rl_repo')

import concourse.bass as bass
import concourse.tile as tile
import concourse.mybir as mybir
from concourse import bacc
from concourse.masks import make_identity

F32 = mybir.dt.float32
BF16 = mybir.dt.bfloat16
I16 = mybir.dt.int16
ALU = mybir.AluOpType
ACTF = mybir.ActivationFunctionType
P = 128


class Cfg:
    def __init__(self, NU=100000, NM=100000, R=5, E=1000000, IN=64, BAS=4,
                 NCORES=8):
        assert NU == NM
        self.NU, self.NM, self.R, self.E, self.IN, self.BAS = NU, NM, R, E, IN, BAS
        self.MPR = 16
        self.MSG = self.MPR * R              # 80
        self.OUT = 64
        self.NCORES = NCORES
        self.NSH = NU // NCORES              # dsts per device
        self.TP = -(-self.NSH // P)          # dst tiles (98)
        self.UN = 8                          # gpsimd units
        self.SLAB = 6400                     # srcs per unit-slab
        self.NG = 2                          # slab groups (16 slabs total)
        self.NELEM = self.R * self.SLAB      # 32000 table elems per partition
        self.NUP = self.SLAB * self.UN * self.NG   # padded src count 102400
        self.NI = 4096                       # ap_gather rows per call


# ----------------------------------------------------------------- host prep

def build_plan(cfg, edge_user, edge_movie):
    """Bin edges per (d, g, unit, tile, rating); pad each (t, r) run to a
    128-multiple of the max count across (device, unit).

    Returns:
      chunks: per (d, g): list of (t, r, first, last) per 128-row chunk
      rowsdg: per (d, g): padded rows per unit
      gidx:  per-device int16 [128, G16] idx blocks (concat over d, g)
      dl8:   per-device f32  [128, CC*8] dloc blocks (-1 = padding)
    """
    NC, UN, TP, R = cfg.NCORES, cfg.UN, cfg.TP, cfg.R
    NSH, SLAB = cfg.NSH, cfg.SLAB
    eu = np.asarray(edge_user)
    em = np.asarray(edge_movie)

    chunks_all = []
    rowsdg = []
    gparts = [[] for _ in range(NC)]
    dparts = [[] for _ in range(NC)]
    for d in range(2):
        src_all, dst_all = (eu, em) if d == 0 else (em, eu)
        src = src_all.reshape(-1).astype(np.int64)
        dst = dst_all.reshape(-1).astype(np.int64)
        rr = np.repeat(np.arange(R, dtype=np.int64), cfg.E)
        c = dst // NSH
        ld = dst % NSH
        t = ld // P
        dl = (ld % P).astype(np.float32)
        slab = src // SLAB
        g = slab // UN
        k = slab % UN
        s = src % SLAB
        idx = (rr * SLAB + s).astype(np.int16)
        for gv in range(2):
            m = g == gv
            key = ((c[m] * UN + k[m]) * TP + t[m]) * R + rr[m]
            cnt = np.bincount(key, minlength=NC * UN * TP * R)
            cnt = cnt.reshape(NC, UN, TP, R)
            nch = -(-cnt.max(axis=(0, 1)) // P)          # [TP, R] chunks
            L = nch * P
            Lf = L.reshape(-1)
            base = np.concatenate([[0], np.cumsum(Lf)[:-1]]).reshape(TP, R)
            rows = int(Lf.sum())
            rowsdg.append(rows)
            # rank within (c,k,t,r)
            order = np.argsort(key, kind='stable')
            ko = key[order]
            bnd = np.flatnonzero(np.diff(ko, prepend=-1))
            rank = np.arange(ko.size) - np.repeat(
                bnd, np.diff(np.append(bnd, ko.size)))
            inv = np.empty_like(order)
            inv[order] = np.arange(order.size)
            rank = rank[inv]
            pos = base[t[m], rr[m]] + rank
            gs = np.zeros((NC, UN, rows), np.int16)
            dv = np.full((NC, UN, rows), -1.0, np.float32)
            gs[c[m], k[m], pos] = idx[m]
            dv[c[m], k[m], pos] = dl[m]
            for cc in range(NC):
                gb = np.zeros((P, rows // 16), np.int16)
                db = np.zeros((P, (rows // P) * UN), np.float32)
                dbv = db.reshape(P, rows // P, UN)
                for kk in range(UN):
                    gb[16 * kk:16 * kk + 16] = \
                        gs[cc, kk].reshape(-1, 16).T
                    dbv[:, :, kk] = dv[cc, kk].reshape(-1, P).T
                gparts[cc].append(gb)
                dparts[cc].append(db)
            ch = []
            for tt in range(TP):
                for r in range(R):
                    n = int(nch[tt, r])
                    for j in range(n):
                        ch.append((tt, r, j == 0, j == n - 1))
            chunks_all.append(ch)
    gidx = [np.concatenate(gp, axis=1) for gp in gparts]
    dl8 = [np.concatenate(dp, axis=1) for dp in dparts]
    return chunks_all, rowsdg, gidx, dl8


# ------------------------------------------------------------- numpy model

def model(cfg, chunks_all, rowsdg, gidx, dl8, inputs):
    """Numpy mirror of the device program (f32, no bf16 rounding)."""
    import ml_dtypes
    BF = ml_dtypes.bfloat16
    R, NSH, TP, SLAB, UN = cfg.R, cfg.NSH, cfg.TP, cfg.SLAB, cfg.UN
    W = np.einsum('rb,bio->rio', np.asarray(inputs['att']),
                  np.asarray(inputs['basis'])).astype(np.float32)
    fc_w = np.asarray(inputs['fc_w'])
    fc_b = np.asarray(inputs['fc_b'])
    xfull = np.zeros((2, cfg.NUP, cfg.MSG), np.float32)
    for side, (f, cj) in enumerate((('ufeat', 'cj_user'), ('ifeat', 'cj_movie'))):
        fc = (np.asarray(inputs[f]) * np.asarray(inputs[cj])).astype(BF)
        for r in range(R):
            xfull[side, :cfg.NU, 16 * r:16 * r + 16] = \
                fc.astype(np.float32) @ W[r].astype(BF).astype(np.float32)
    ci = [np.asarray(inputs['ci_movie']), np.asarray(inputs['ci_user'])]

    u_out = np.zeros((cfg.NU, cfg.OUT), np.float32)
    m_out = np.zeros((cfg.NM, cfg.OUT), np.float32)
    for c in range(cfg.NCORES):
        g16o = 0
        cco = 0
        for d in range(2):
            yacc = np.zeros((TP * P, cfg.MSG), np.float32)
            for g in range(2):
                ch = chunks_all[d * 2 + g]
                rows = rowsdg[d * 2 + g]
                gb = gidx[c][:, g16o:g16o + rows // 16]
                db = dl8[c][:, cco:cco + (rows // P) * UN].reshape(
                    P, rows // P, UN)
                g16o += rows // 16
                cco += (rows // P) * UN
                trow = np.repeat([t for (t, r, _, _) in ch], P)
                rrow = np.repeat([r for (t, r, _, _) in ch], P)
                for k in range(UN):
                    idxs = gb[16 * k:16 * k + 16].T.reshape(-1)  # [rows]
                    dls = db[:, :, k].T.reshape(-1)
                    msk = dls >= 0
                    base = (8 * g + k) * SLAB
                    elem = idxs.astype(np.int64)
                    s = elem % SLAB
                    rv = elem // SLAB
                    val = np.zeros((rows, 16), np.float32)
                    sel = xfull[d, base + s]                      # [rows, 80]
                    val = sel[np.arange(rows)[:, None],
                              (rv * 16)[:, None] + np.arange(16)[None, :]]
                    tgt = trow * P + dls.astype(np.int64)
                    col = rrow * 16
                    np.add.at(yacc, (tgt[msk][:, None],
                                     (col[msk][:, None] + np.arange(16))),
                              val[msk])
            cish = np.zeros((TP * P, 1), np.float32)
            cish[:NSH] = ci[d][c * NSH:(c + 1) * NSH]
            z = np.maximum(yacc * cish, 0.0) @ fc_w.T + fc_b
            if d == 0:
                m_out[c * NSH:(c + 1) * NSH] = z[:NSH]
            else:
                u_out[c * NSH:(c + 1) * NSH] = z[:NSH]
    return u_out, m_out


# ---------------------------------------------------------- device program

def build_program(cfg, chunks_all, rowsdg, g16cols, cccols, num_devices):
    nc = bacc.Bacc("TRN2", target_bir_lowering=False, debug=False,
                   num_devices=num_devices)
    NU, IN, R, BAS = cfg.NU, cfg.IN, cfg.R, cfg.BAS
    TP, UN, SLAB, NELEM, MSG, NI = (cfg.TP, cfg.UN, cfg.SLAB, cfg.NELEM,
                                    cfg.MSG, cfg.NI)

    ufeat = nc.dram_tensor("ufeat", (NU, IN), F32, kind="ExternalInput")
    ifeat = nc.dram_tensor("ifeat", (NU, IN), F32, kind="ExternalInput")
    cj_u = nc.dram_tensor("cj_u", (NU, 1), F32, kind="ExternalInput")
    cj_m = nc.dram_tensor("cj_m", (NU, 1), F32, kind="ExternalInput")
    ci_sh = nc.dram_tensor("ci_sh", (2 * TP * P, 1), F32, kind="ExternalInput")
    attT = nc.dram_tensor("attT", (BAS, R), F32, kind="ExternalInput")
    basis2 = nc.dram_tensor("basis2", (BAS, IN * 16), F32, kind="ExternalInput")
    fc_w = nc.dram_tensor("fc_w", (64, MSG), F32, kind="ExternalInput")
    fc_b = nc.dram_tensor("fc_b", (1, 64), F32, kind="ExternalInput")
    iota_d = nc.dram_tensor("iota128", (P, P), BF16, kind="ExternalInput")
    gidx = nc.dram_tensor("gidx", (P, g16cols), I16, kind="ExternalInput")
    dl8 = nc.dram_tensor("dl8", (P, cccols), BF16, kind="ExternalInput")

    m_out = nc.dram_tensor("m_out", (TP * P, 64), F32, kind="ExternalOutput")
    u_out = nc.dram_tensor("u_out", (TP * P, 64), F32, kind="ExternalOutput")

    wscr = nc.dram_tensor("wscr", (R, IN * 16), F32, kind="Internal")
    fsrc16 = nc.dram_tensor("fsrc16", (2 * cfg.NUP, IN), BF16, kind="Internal")
    tblst = nc.dram_tensor("tblst", (P, NELEM), F32, kind="Internal")

    with tile.TileContext(nc) as tc:
        with tc.tile_pool(name="const", bufs=1) as pool:
            # ---------------- constants ----------------
            pp_ctx = tc.tile_pool(name="cpsum", bufs=2, space="PSUM")
            pp = pp_ctx.__enter__()
            ident = pool.tile([P, P], F32)
            make_identity(nc, ident[:])
            ident16 = pool.tile([P, P], BF16)
            make_identity(nc, ident16[:])

            with tc.tile_pool(name="w0", bufs=1) as wp:
                at = wp.tile([BAS, R], F32)
                bs = wp.tile([BAS, IN * 16], F32)
                nc.sync.dma_start(out=at[:], in_=attT.ap()[:])
                nc.sync.dma_start(out=bs[:], in_=basis2.ap()[:])
                w5 = wp.tile([R, IN * 16], F32)
                half = IN * 16 // 2
                for h in range(2):
                    ps = pp.tile([R, half], F32, space="PSUM", tag="w5ps")
                    nc.tensor.matmul(out=ps[:], lhsT=at[:],
                                     rhs=bs[:, h * half:(h + 1) * half],
                                     start=True, stop=True)
                    nc.scalar.copy(out=w5[:, h * half:(h + 1) * half], in_=ps[:])
                nc.sync.dma_start(out=wscr.ap()[:], in_=w5[:])
            w64 = pool.tile([IN, R, 16], F32)
            nc.sync.dma_start(
                out=w64[:], in_=wscr.ap()[:].rearrange("r (k o) -> k r o", k=IN))
            wallb = pool.tile([IN, R * 16], BF16)
            nc.scalar.copy(out=wallb[:],
                           in_=w64[:].rearrange("k r o -> k (r o)"))

            fcw = pool.tile([64, MSG], F32)
            nc.sync.dma_start(out=fcw[:], in_=fc_w.ap()[:])
            psT = pp.tile([MSG, 64], F32, space="PSUM", tag="fcT")
            nc.tensor.transpose(out=psT[:], in_=fcw[:], identity=ident[:64, :64])
            fcwT = pool.tile([MSG, 64], F32)
            nc.scalar.copy(out=fcwT[:], in_=psT[:])
            fcb = pool.tile([P, 64], F32)
            nc.sync.dma_start(out=fcb[:], in_=fc_b.ap()[:].to_broadcast((P, 64)))

            cisb = pool.tile([P, 2 * TP], F32)
            nc.sync.dma_start(
                out=cisb[:],
                in_=ci_sh.ap()[:].rearrange("(t p) o -> p (t o)", p=P))
            iota = pool.tile([P, 1, P], BF16)
            nc.sync.dma_start(out=iota[:, 0, :], in_=iota_d.ap()[:])
            pp_ctx.__exit__(None, None, None)

            # ---------------- stage fsrc16 = (feat*cj) bf16 ----------------
            # Side 0 staged up front; side 1 staged during d=0's first
            # gather phase so its DMAs/DVE hide under the Pool gathers.
            GT = 8

            def stage_side(side, feat, cj):
                with tc.tile_pool(name=f"p1{side}", bufs=3) as p1:
                    starts = list(range(0, NU - GT * P + 1, GT * P))
                    if NU % (GT * P):
                        starts.append(NU - GT * P)
                    for g0 in starts:
                        ft = p1.tile([P, GT, IN], F32, tag="ft")
                        cjt = p1.tile([P, GT, 1], F32, tag="cj")
                        nc.sync.dma_start(
                            out=ft[:], in_=feat.ap()[g0:g0 + GT * P].rearrange(
                                "(p a) d -> p a d", p=P))
                        nc.sync.dma_start(
                            out=cjt[:], in_=cj.ap()[g0:g0 + GT * P].rearrange(
                                "(p a) d -> p a d", p=P))
                        sc = p1.tile([P, GT, IN], BF16, tag="sc")
                        nc.vector.tensor_tensor(
                            out=sc[:], in0=ft[:],
                            in1=cjt[:].to_broadcast((P, GT, IN)),
                            op=ALU.mult)
                        ofs = side * cfg.NUP + g0
                        nc.sync.dma_start(
                            out=fsrc16.ap()[ofs:ofs + GT * P]
                                .rearrange("(p a) d -> p a d", p=P),
                            in_=sc[:])

            stage_side(0, ufeat, cj_u)

            # ---------------- main: per direction ----------------
            # Tables are prebuilt into DRAM (tblst) during the PREVIOUS
            # phase's gathers; each phase boundary is just one 16MB DMA
            # load tblst -> tbl instead of a serialized rebuild.
            from contextlib import ExitStack

            def build_stage(dd, gg, tb, tbx, tbps):
                for k in range(UN):
                    base = dd * cfg.NUP + (UN * gg + k) * SLAB
                    for hh in range(2):
                        half = SLAB // 2
                        xts = tbx.tile([MSG, half], F32, tag="xts")
                        for c0 in range(0, half, 512):
                            w = min(512, half - c0)
                            fc4 = tb.tile([P, 4, IN], BF16, tag="fc4")
                            nc.sync.dma_start(
                                out=fc4[:, :w // P, :],
                                in_=fsrc16.ap()[
                                    base + hh * half + c0:
                                    base + hh * half + c0 + w]
                                    .rearrange("(a p) d -> p a d", p=P))
                            ft4 = tb.tile([IN, 4, P], BF16, tag="ft4")
                            for j in range(w // P):
                                fps = tbps.tile([IN, P], BF16, space="PSUM",
                                                tag="fps")
                                nc.tensor.transpose(
                                    out=fps[:], in_=fc4[:, j, :],
                                    identity=ident16[:])
                                nc.vector.tensor_copy(
                                    out=ft4[:, j, :], in_=fps[:])
                            xps = tbps.tile([MSG, 512], F32, space="PSUM",
                                            tag="xps")
                            nc.tensor.matmul(
                                out=xps[:, :w],
                                lhsT=wallb[:],
                                rhs=ft4[:].rearrange(
                                    "f a p -> f (a p)")[:, :w],
                                start=True, stop=True)
                            nc.vector.tensor_copy(
                                out=xts[:, c0:c0 + w], in_=xps[:, :w])
                        for r in range(R):
                            nc.sync.dma_start(
                                out=tblst.ap()[
                                    16 * k:16 * k + 16,
                                    r * SLAB + hh * (SLAB // 2):
                                    r * SLAB + (hh + 1) * (SLAB // 2)],
                                in_=xts[16 * r:16 * r + 16, :])

            g16ofs = 0
            ccofs = 0
            with tc.tile_pool(name="tblp", bufs=1) as tblp, \
                 tc.tile_pool(name="yaccp", bufs=1) as yp:
                tbl = tblp.tile([P, NELEM, 1], F32)
                with tc.tile_pool(name="tbi", bufs=2) as tb0, \
                     tc.tile_pool(name="tbxi", bufs=1) as tbx0, \
                     tc.tile_pool(name="tbpsi", bufs=2, space="PSUM") as tbps0:
                    build_stage(0, 0, tb0, tbx0, tbps0)
                for d in range(2):
                    yacc = yp.tile([P, TP, MSG], BF16, tag="yacc")
                    nc.vector.memset(yacc[:], 0.0)
                    for g in range(2):
                        ph = 2 * d + g
                        nc.sync.dma_start(out=tbl[:, :, 0], in_=tblst.ap()[:])
                        bstack = ExitStack()
                        if ph < 3:
                            tb = bstack.enter_context(
                                tc.tile_pool(name=f"tb{ph}", bufs=2))
                            tbx = bstack.enter_context(
                                tc.tile_pool(name=f"tbx{ph}", bufs=1))
                            tbps = bstack.enter_context(
                                tc.tile_pool(name=f"tbps{ph}", bufs=2,
                                             space="PSUM"))
                            build_stage((ph + 1) // 2, (ph + 1) % 2,
                                        tb, tbx, tbps)
                        if d == 0 and g == 0:
                            stage_side(1, ifeat, cj_m)
                        # ---- gather + segment-sum for (d, g) ----
                        chunksl = chunks_all[d * 2 + g]
                        rows = rowsdg[d * 2 + g]
                        with tc.tile_pool(name="io", bufs=2) as iop, \
                             tc.tile_pool(name="go", bufs=1) as gop, \
                             tc.tile_pool(name="gb", bufs=1) as gbp, \
                             tc.tile_pool(name="oh", bufs=3) as ohp, \
                             tc.tile_pool(name="ts", bufs=3) as tsp, \
                             tc.tile_pool(name="mps", bufs=2,
                                          space="PSUM") as mps:
                            pos = 0
                            psy = None
                            for a0 in range(0, rows, NI):
                                ni = min(NI, rows - a0)
                                gi = iop.tile([P, NI // 16], I16, tag="gi")
                                nc.sync.dma_start(
                                    out=gi[:, :ni // 16],
                                    in_=gidx.ap()[:, g16ofs + a0 // 16:
                                                  g16ofs + (a0 + ni) // 16])
                                dlt = iop.tile([P, NI // P, UN, 1], BF16,
                                               tag="dlt")
                                nc.sync.dma_start(
                                    out=dlt[:, :ni // P, :, :],
                                    in_=dl8.ap()[:, ccofs + (a0 // P) * UN:
                                                 ccofs + ((a0 + ni) // P) * UN]
                                        .rearrange("p (c k one) -> p c k one",
                                                   k=UN, one=1))
                                go = gop.tile([P, NI, 1], F32, tag="go")
                                nc.gpsimd.ap_gather(
                                    go[:, :ni, :], tbl[:], gi[:, :ni // 16],
                                    channels=P, num_elems=NELEM, d=1,
                                    num_idxs=ni)
                                gb = gbp.tile([P, NI], BF16, tag="gbc")
                                nc.vector.tensor_copy(out=gb[:, :ni],
                                                      in_=go[:, :ni, 0])
                                for local in range(ni // P):
                                    t, r, first, last = chunksl[pos]
                                    pos += 1
                                    tps = mps.tile([P, P], BF16, space="PSUM",
                                                   tag="tps")
                                    nc.tensor.transpose(
                                        out=tps[:],
                                        in_=gb[:, local * P:(local + 1) * P],
                                        identity=ident16[:])
                                    tsb = tsp.tile([P, P], BF16, tag="tsb")
                                    nc.vector.tensor_copy(out=tsb[:], in_=tps[:])
                                    oh = ohp.tile([P, UN, P], BF16, tag="oh")
                                    nc.vector.tensor_tensor(
                                        out=oh[:],
                                        in0=dlt[:, local, :, :].to_broadcast(
                                            (P, UN, P)),
                                        in1=iota[:, 0:1, :].to_broadcast(
                                            (P, UN, P)),
                                        op=ALU.is_equal)
                                    if first:
                                        psy = mps.tile([P, 16], F32,
                                                       space="PSUM", tag="psy")
                                    for k in range(UN):
                                        nc.tensor.matmul(
                                            out=psy[:],
                                            lhsT=oh[:, k, :],
                                            rhs=tsb[:, 16 * k:16 * k + 16],
                                            start=(first and k == 0),
                                            stop=(last and k == UN - 1))
                                    if last:
                                        ys = yacc[:, t, r * 16:(r + 1) * 16]
                                        nc.vector.tensor_tensor(
                                            out=ys, in0=ys, in1=psy[:],
                                            op=ALU.add)
                        bstack.close()
                        g16ofs += rows // 16
                        ccofs += (rows // P) * UN
                    # ---------------- transform ----------------
                    with tc.tile_pool(name="p3", bufs=3) as p3, \
                         tc.tile_pool(name="p3ps", bufs=2, space="PSUM") as p3p:
                        for t in range(TP):
                            msg = p3.tile([P, MSG], F32, tag="msg")
                            nc.scalar.activation(
                                out=msg[:], in_=yacc[:, t, :],
                                func=ACTF.Relu,
                                scale=cisb[:, d * TP + t: d * TP + t + 1])
                            psmT = p3p.tile([MSG, P], F32, space="PSUM",
                                            tag="psmT")
                            nc.tensor.transpose(out=psmT[:], in_=msg[:],
                                                identity=ident[:])
                            msgT = p3.tile([MSG, P], F32, tag="msgT")
                            nc.vector.tensor_copy(out=msgT[:], in_=psmT[:])
                            fcp = p3p.tile([P, 64], F32, space="PSUM",
                                           tag="fcp")
                            nc.tensor.matmul(
                                out=fcp[:], lhsT=msgT[:], rhs=fcwT[:],
                                start=True, stop=True)
                            osb = p3.tile([P, 64], F32, tag="osb")
                            nc.vector.tensor_tensor(out=osb[:], in0=fcp[:],
                                                    in1=fcb[:], op=ALU.add)
                            dst = m_out if d == 0 else u_out
                            nc.sync.dma_start(
                                out=dst.ap()[t * P:(t + 1) * P], in_=osb[:])
    nc.compile()
    return nc


# ----------------------------------------------------------------- kernel

def make_in_maps(cfg, gidx, dl8, inputs):
    import ml_dtypes
    ins = {k: np.asarray(v) for k, v in inputs.items()}
    iota = np.tile(np.arange(P, dtype=ml_dtypes.bfloat16), (P, 1))
    dl8 = [d.astype(ml_dtypes.bfloat16) for d in dl8]
    base = dict(
        ufeat=ins['ufeat'], ifeat=ins['ifeat'],
        cj_u=ins['cj_user'], cj_m=ins['cj_movie'],
        attT=np.ascontiguousarray(ins['att'].T),
        basis2=ins['basis'].reshape(cfg.BAS, cfg.IN * 16).copy(),
        fc_w=ins['fc_w'], fc_b=ins['fc_b'].reshape(1, 64).copy(),
        iota128=iota,
    )
    in_maps = []
    for c in range(cfg.NCORES):
        ci = np.zeros((2 * cfg.TP * P, 1), np.float32)
        ci[:cfg.NSH] = ins['ci_movie'][c * cfg.NSH:(c + 1) * cfg.NSH]
        ci[cfg.TP * P:cfg.TP * P + cfg.NSH] = \
            ins['ci_user'][c * cfg.NSH:(c + 1) * cfg.NSH]
        in_maps.append({**base, 'ci_sh': ci, 'gidx': gidx[c], 'dl8': dl8[c]})
    return in_maps


def assemble(cfg, results):
    u = np.concatenate([results[c]['u_out'][:cfg.NSH]
                        for c in range(cfg.NCORES)])
    m = np.concatenate([results[c]['m_out'][:cfg.NSH]
                        for c in range(cfg.NCORES)])
    return u, m


def kernel(**inputs):
    from concourse import bass_utils
    cfg = Cfg()
    chunks_all, rowsdg, gidx, dl8 = build_plan(cfg, inputs['edge_user'],
                                               inputs['edge_movie'])
    nc = build_program(cfg, chunks_all, rowsdg, gidx[0].shape[1],
                       dl8[0].shape[1], cfg.NCORES)
    in_maps = make_in_maps(cfg, gidx, dl8, inputs)
    res = bass_utils.run_bass_kernel_spmd(nc, in_maps,
                                          core_ids=list(range(cfg.NCORES)))
    return assemble(cfg, res.results)


# revision 23
# speedup vs baseline: 1.1529x; 1.1529x over previous
"""GCMC layer on trn2 — v2: ap_gather (GPSIMD free-dim gather) + one-hot PE
segment-sum in transformed (16-lane) message space.

Design (per device, dst-sharded: device c owns dst nodes [c*NSH, (c+1)*NSH)):
  - Transform-first: x_r = (feat*cj) @ W_r  ([N,16] per rating) so each edge
    only moves 16 lanes. Table per (side, g): SBUF [128, 32000] f32 where
    partition 16k+j = lane j of slab (8g+k) (slab=6400 srcs), elem r*6400+s.
  - Edges binned per (d, g, unit k=src slab, dst-tile t, rating r); each
    (t, r) run padded to a 128-multiple of the max count over (device, unit)
    so the SPMD program is uniform. Unit streams are position-aligned: at any
    chunk all 8 units are in the same (t, r) run.
  - nc.gpsimd.ap_gather pulls per-edge x_r lanes from the SBUF table
    (per-unit int16 idx lists; Pool engine, ~6-9ns/idx/unit, 8 units in
    parallel — replaces dma_gather's ~8.6ns/row serial descriptor gen).
  - Per 128-row position: PE transpose -> T [128 e, 128 (k,j)]; DVE builds 8
    one-hots from dloc codes; 8 matmuls accumulate psum_y[dst,16] per run;
    DVE flushes into yacc [128, TP, 80] bf16.
  - Table build: stage fsrc16=(feat*cj) bf16 in DRAM; per slab DMA-transpose
    -> FT [64, 6400]; xT = Wall^T @ FT on PE; DVE copy psum->xTslab f32;
    5 HWDGE DMAs partition-remap xTslab -> table unit block.
  - Transform per (d, tile): relu(msg*ci) on ACT, PE transpose, f32 fc matmul
    + bias, dense store.
Host only bins/sorts indices and packs int16 idx + f32 dloc blocks.
"""
import sys
import numpy as np

sys.path.insert(0, '/opt/trn_rl_repo')

import concourse.bass as bass
import concourse.tile as tile
import concourse.mybir as mybir
from concourse import bacc
from concourse.masks import make_identity

F32 = mybir.dt.float32
BF16 = mybir.dt.bfloat16
I16 = mybir.dt.int16
ALU = mybir.AluOpType
ACTF = mybir.ActivationFunctionType
P = 128


class Cfg:
    def __init__(self, NU=100000, NM=100000, R=5, E=1000000, IN=64, BAS=4,
                 NCORES=8):
        assert NU == NM
        self.NU, self.NM, self.R, self.E, self.IN, self.BAS = NU, NM, R, E, IN, BAS
        self.MPR = 16
        self.MSG = self.MPR * R              # 80
        self.OUT = 64
        self.NCORES = NCORES
        self.NSH = NU // NCORES              # dsts per device
        self.TP = -(-self.NSH // P)          # dst tiles (98)
        self.UN = 8                          # gpsimd units
        self.SLAB = 6400                     # srcs per unit-slab
        self.NG = 2                          # slab groups (16 slabs total)
        self.NELEM = self.R * self.SLAB      # 32000 table elems per partition
        self.NUP = self.SLAB * self.UN * self.NG   # padded src count 102400
        self.NI = 5120                       # ap_gather rows per call


# ----------------------------------------------------------------- host prep

def build_plan(cfg, edge_user, edge_movie):
    """Bin edges per (d, g, unit, tile, rating); pad each (t, r) run to a
    128-multiple of the max count across (device, unit).

    Returns:
      chunks: per (d, g): list of (t, r, first, last) per 128-row chunk
      rowsdg: per (d, g): padded rows per unit
      gidx:  per-device int16 [128, G16] idx blocks (concat over d, g)
      dl8:   per-device f32  [128, CC*8] dloc blocks (-1 = padding)
    """
    NC, UN, TP, R = cfg.NCORES, cfg.UN, cfg.TP, cfg.R
    NSH, SLAB = cfg.NSH, cfg.SLAB
    eu = np.asarray(edge_user)
    em = np.asarray(edge_movie)

    chunks_all = []
    rowsdg = []
    gparts = [[] for _ in range(NC)]
    dparts = [[] for _ in range(NC)]
    for d in range(2):
        src_all, dst_all = (eu, em) if d == 0 else (em, eu)
        src = src_all.reshape(-1).astype(np.int64)
        dst = dst_all.reshape(-1).astype(np.int64)
        rr = np.repeat(np.arange(R, dtype=np.int64), cfg.E)
        c = dst // NSH
        ld = dst % NSH
        t = ld // P
        dl = (ld % P).astype(np.float32)
        slab = src // SLAB
        g = slab // UN
        k = slab % UN
        s = src % SLAB
        idx = (rr * SLAB + s).astype(np.int16)
        for gv in range(2):
            m = g == gv
            key = ((c[m] * UN + k[m]) * TP + t[m]) * R + rr[m]
            cnt = np.bincount(key, minlength=NC * UN * TP * R)
            cnt = cnt.reshape(NC, UN, TP, R)
            nch = -(-cnt.max(axis=(0, 1)) // P)          # [TP, R] chunks
            L = nch * P
            Lf = L.reshape(-1)
            base = np.concatenate([[0], np.cumsum(Lf)[:-1]]).reshape(TP, R)
            rows = int(Lf.sum())
            rowsdg.append(rows)
            # rank within (c,k,t,r)
            order = np.argsort(key, kind='stable')
            ko = key[order]
            bnd = np.flatnonzero(np.diff(ko, prepend=-1))
            rank = np.arange(ko.size) - np.repeat(
                bnd, np.diff(np.append(bnd, ko.size)))
            inv = np.empty_like(order)
            inv[order] = np.arange(order.size)
            rank = rank[inv]
            pos = base[t[m], rr[m]] + rank
            gs = np.zeros((NC, UN, rows), np.int16)
            dv = np.full((NC, UN, rows), -1.0, np.float32)
            gs[c[m], k[m], pos] = idx[m]
            dv[c[m], k[m], pos] = dl[m]
            for cc in range(NC):
                gb = np.zeros((P, rows // 16), np.int16)
                db = np.zeros((P, (rows // P) * UN), np.float32)
                dbv = db.reshape(P, rows // P, UN)
                for kk in range(UN):
                    gb[16 * kk:16 * kk + 16] = \
                        gs[cc, kk].reshape(-1, 16).T
                    dbv[:, :, kk] = dv[cc, kk].reshape(-1, P).T
                gparts[cc].append(gb)
                dparts[cc].append(db)
            ch = []
            for tt in range(TP):
                for r in range(R):
                    n = int(nch[tt, r])
                    for j in range(n):
                        ch.append((tt, r, j == 0, j == n - 1))
            chunks_all.append(ch)
    gidx = [np.concatenate(gp, axis=1) for gp in gparts]
    dl8 = [np.concatenate(dp, axis=1) for dp in dparts]
    return chunks_all, rowsdg, gidx, dl8


# ------------------------------------------------------------- numpy model

def model(cfg, chunks_all, rowsdg, gidx, dl8, inputs):
    """Numpy mirror of the device program (f32, no bf16 rounding)."""
    import ml_dtypes
    BF = ml_dtypes.bfloat16
    R, NSH, TP, SLAB, UN = cfg.R, cfg.NSH, cfg.TP, cfg.SLAB, cfg.UN
    W = np.einsum('rb,bio->rio', np.asarray(inputs['att']),
                  np.asarray(inputs['basis'])).astype(np.float32)
    fc_w = np.asarray(inputs['fc_w'])
    fc_b = np.asarray(inputs['fc_b'])
    xfull = np.zeros((2, cfg.NUP, cfg.MSG), np.float32)
    for side, (f, cj) in enumerate((('ufeat', 'cj_user'), ('ifeat', 'cj_movie'))):
        fc = (np.asarray(inputs[f]) * np.asarray(inputs[cj])).astype(BF)
        for r in range(R):
            xfull[side, :cfg.NU, 16 * r:16 * r + 16] = \
                fc.astype(np.float32) @ W[r].astype(BF).astype(np.float32)
    ci = [np.asarray(inputs['ci_movie']), np.asarray(inputs['ci_user'])]

    u_out = np.zeros((cfg.NU, cfg.OUT), np.float32)
    m_out = np.zeros((cfg.NM, cfg.OUT), np.float32)
    for c in range(cfg.NCORES):
        g16o = 0
        cco = 0
        for d in range(2):
            yacc = np.zeros((TP * P, cfg.MSG), np.float32)
            for g in range(2):
                ch = chunks_all[d * 2 + g]
                rows = rowsdg[d * 2 + g]
                gb = gidx[c][:, g16o:g16o + rows // 16]
                db = dl8[c][:, cco:cco + (rows // P) * UN].reshape(
                    P, rows // P, UN)
                g16o += rows // 16
                cco += (rows // P) * UN
                trow = np.repeat([t for (t, r, _, _) in ch], P)
                rrow = np.repeat([r for (t, r, _, _) in ch], P)
                for k in range(UN):
                    idxs = gb[16 * k:16 * k + 16].T.reshape(-1)  # [rows]
                    dls = db[:, :, k].T.reshape(-1)
                    msk = dls >= 0
                    base = (8 * g + k) * SLAB
                    elem = idxs.astype(np.int64)
                    s = elem % SLAB
                    rv = elem // SLAB
                    val = np.zeros((rows, 16), np.float32)
                    sel = xfull[d, base + s]                      # [rows, 80]
                    val = sel[np.arange(rows)[:, None],
                              (rv * 16)[:, None] + np.arange(16)[None, :]]
                    tgt = trow * P + dls.astype(np.int64)
                    col = rrow * 16
                    np.add.at(yacc, (tgt[msk][:, None],
                                     (col[msk][:, None] + np.arange(16))),
                              val[msk])
            cish = np.zeros((TP * P, 1), np.float32)
            cish[:NSH] = ci[d][c * NSH:(c + 1) * NSH]
            z = np.maximum(yacc * cish, 0.0) @ fc_w.T + fc_b
            if d == 0:
                m_out[c * NSH:(c + 1) * NSH] = z[:NSH]
            else:
                u_out[c * NSH:(c + 1) * NSH] = z[:NSH]
    return u_out, m_out


# ---------------------------------------------------------- device program

def build_program(cfg, chunks_all, rowsdg, g16cols, cccols, num_devices):
    nc = bacc.Bacc("TRN2", target_bir_lowering=False, debug=False,
                   num_devices=num_devices)
    NU, IN, R, BAS = cfg.NU, cfg.IN, cfg.R, cfg.BAS
    TP, UN, SLAB, NELEM, MSG, NI = (cfg.TP, cfg.UN, cfg.SLAB, cfg.NELEM,
                                    cfg.MSG, cfg.NI)

    ufeat = nc.dram_tensor("ufeat", (NU, IN), F32, kind="ExternalInput")
    ifeat = nc.dram_tensor("ifeat", (NU, IN), F32, kind="ExternalInput")
    cj_u = nc.dram_tensor("cj_u", (NU, 1), F32, kind="ExternalInput")
    cj_m = nc.dram_tensor("cj_m", (NU, 1), F32, kind="ExternalInput")
    ci_sh = nc.dram_tensor("ci_sh", (2 * TP * P, 1), F32, kind="ExternalInput")
    attT = nc.dram_tensor("attT", (BAS, R), F32, kind="ExternalInput")
    basis2 = nc.dram_tensor("basis2", (BAS, IN * 16), F32, kind="ExternalInput")
    fc_w = nc.dram_tensor("fc_w", (64, MSG), F32, kind="ExternalInput")
    fc_b = nc.dram_tensor("fc_b", (1, 64), F32, kind="ExternalInput")
    iota_d = nc.dram_tensor("iota128", (P, P), BF16, kind="ExternalInput")
    gidx = nc.dram_tensor("gidx", (P, g16cols), I16, kind="ExternalInput")
    dl8 = nc.dram_tensor("dl8", (P, cccols), BF16, kind="ExternalInput")

    m_out = nc.dram_tensor("m_out", (TP * P, 64), F32, kind="ExternalOutput")
    u_out = nc.dram_tensor("u_out", (TP * P, 64), F32, kind="ExternalOutput")

    wscr = nc.dram_tensor("wscr", (R, IN * 16), F32, kind="Internal")
    fsrc16 = nc.dram_tensor("fsrc16", (2 * cfg.NUP, IN), BF16, kind="Internal")
    tblst = nc.dram_tensor("tblst", (P, NELEM), F32, kind="Internal")

    with tile.TileContext(nc) as tc:
        with tc.tile_pool(name="const", bufs=1) as pool:
            # ---------------- constants ----------------
            pp_ctx = tc.tile_pool(name="cpsum", bufs=2, space="PSUM")
            pp = pp_ctx.__enter__()
            ident = pool.tile([P, P], F32)
            make_identity(nc, ident[:])
            ident16 = pool.tile([P, P], BF16)
            make_identity(nc, ident16[:])

            with tc.tile_pool(name="w0", bufs=1) as wp:
                at = wp.tile([BAS, R], F32)
                bs = wp.tile([BAS, IN * 16], F32)
                nc.sync.dma_start(out=at[:], in_=attT.ap()[:])
                nc.sync.dma_start(out=bs[:], in_=basis2.ap()[:])
                w5 = wp.tile([R, IN * 16], F32)
                half = IN * 16 // 2
                for h in range(2):
                    ps = pp.tile([R, half], F32, space="PSUM", tag="w5ps")
                    nc.tensor.matmul(out=ps[:], lhsT=at[:],
                                     rhs=bs[:, h * half:(h + 1) * half],
                                     start=True, stop=True)
                    nc.scalar.copy(out=w5[:, h * half:(h + 1) * half], in_=ps[:])
                nc.sync.dma_start(out=wscr.ap()[:], in_=w5[:])
            w64 = pool.tile([IN, R, 16], F32)
            nc.sync.dma_start(
                out=w64[:], in_=wscr.ap()[:].rearrange("r (k o) -> k r o", k=IN))
            wallb = pool.tile([IN, R * 16], BF16)
            nc.scalar.copy(out=wallb[:],
                           in_=w64[:].rearrange("k r o -> k (r o)"))

            fcw = pool.tile([64, MSG], F32)
            nc.sync.dma_start(out=fcw[:], in_=fc_w.ap()[:])
            psT = pp.tile([MSG, 64], F32, space="PSUM", tag="fcT")
            nc.tensor.transpose(out=psT[:], in_=fcw[:], identity=ident[:64, :64])
            fcwT = pool.tile([MSG, 64], F32)
            nc.scalar.copy(out=fcwT[:], in_=psT[:])
            fcb = pool.tile([P, 64], F32)
            nc.sync.dma_start(out=fcb[:], in_=fc_b.ap()[:].to_broadcast((P, 64)))

            cisb = pool.tile([P, 2 * TP], F32)
            nc.sync.dma_start(
                out=cisb[:],
                in_=ci_sh.ap()[:].rearrange("(t p) o -> p (t o)", p=P))
            iota = pool.tile([P, 1, P], BF16)
            nc.sync.dma_start(out=iota[:, 0, :], in_=iota_d.ap()[:])
            pp_ctx.__exit__(None, None, None)

            # ---------------- stage fsrc16 = (feat*cj) bf16 ----------------
            # Side 0 staged up front; side 1 staged during d=0's first
            # gather phase so its DMAs/DVE hide under the Pool gathers.
            GT = 8

            def stage_side(side, feat, cj):
                with tc.tile_pool(name=f"p1{side}", bufs=3) as p1:
                    starts = list(range(0, NU - GT * P + 1, GT * P))
                    if NU % (GT * P):
                        starts.append(NU - GT * P)
                    for g0 in starts:
                        ft = p1.tile([P, GT, IN], F32, tag="ft")
                        cjt = p1.tile([P, GT, 1], F32, tag="cj")
                        nc.sync.dma_start(
                            out=ft[:], in_=feat.ap()[g0:g0 + GT * P].rearrange(
                                "(p a) d -> p a d", p=P))
                        nc.sync.dma_start(
                            out=cjt[:], in_=cj.ap()[g0:g0 + GT * P].rearrange(
                                "(p a) d -> p a d", p=P))
                        sc = p1.tile([P, GT, IN], BF16, tag="sc")
                        nc.vector.tensor_tensor(
                            out=sc[:], in0=ft[:],
                            in1=cjt[:].to_broadcast((P, GT, IN)),
                            op=ALU.mult)
                        ofs = side * cfg.NUP + g0
                        nc.sync.dma_start(
                            out=fsrc16.ap()[ofs:ofs + GT * P]
                                .rearrange("(p a) d -> p a d", p=P),
                            in_=sc[:])

            stage_side(0, ufeat, cj_u)
            stage_side(1, ifeat, cj_m)

            # Build one unit-slab of the (dd, gg) table into the DRAM stage.
            # Emitted interleaved between gather calls of the previous phase
            # so PE/DVE slices fit in per-call slack instead of front-loading
            # the engine FIFOs.
            def build_unit(dd, gg, k, tb, tbx, tbps):
                base = dd * cfg.NUP + (UN * gg + k) * SLAB
                for hh in range(2):
                    half = SLAB // 2
                    xts = tbx.tile([MSG, half], F32, tag="xts")
                    for c0 in range(0, half, 512):
                        w = min(512, half - c0)
                        fc4 = tb.tile([P, 4, IN], BF16, tag="fc4")
                        nc.sync.dma_start(
                            out=fc4[:, :w // P, :],
                            in_=fsrc16.ap()[
                                base + hh * half + c0:
                                base + hh * half + c0 + w]
                                .rearrange("(a p) d -> p a d", p=P))
                        ft4 = tb.tile([IN, 4, P], BF16, tag="ft4")
                        for j in range(w // P):
                            fps = tbps.tile([IN, P], BF16, space="PSUM",
                                            tag="fps")
                            nc.tensor.transpose(
                                out=fps[:], in_=fc4[:, j, :],
                                identity=ident16[:])
                            nc.vector.tensor_copy(
                                out=ft4[:, j, :], in_=fps[:])
                        xps = tbps.tile([MSG, 512], F32, space="PSUM",
                                        tag="xps")
                        nc.tensor.matmul(
                            out=xps[:, :w], lhsT=wallb[:],
                            rhs=ft4[:].rearrange("f a p -> f (a p)")[:, :w],
                            start=True, stop=True)
                        nc.vector.tensor_copy(
                            out=xts[:, c0:c0 + w], in_=xps[:, :w])
                    for r in range(R):
                        nc.sync.dma_start(
                            out=tblst.ap()[
                                16 * k:16 * k + 16,
                                r * SLAB + hh * (SLAB // 2):
                                r * SLAB + (hh + 1) * (SLAB // 2)],
                            in_=xts[16 * r:16 * r + 16, :])

            # ---------------- main: per direction ----------------
            g16ofs = 0
            ccofs = 0
            with tc.tile_pool(name="tblp", bufs=1) as tblp, \
                 tc.tile_pool(name="yaccp", bufs=1) as yp:
                tbl = tblp.tile([P, NELEM, 1], F32)
                from contextlib import ExitStack
                with tc.tile_pool(name="tbi", bufs=2) as tb0, \
                     tc.tile_pool(name="tbxi", bufs=1) as tbx0, \
                     tc.tile_pool(name="tbpsi", bufs=2, space="PSUM") as tbps0:
                    for k in range(UN):
                        build_unit(0, 0, k, tb0, tbx0, tbps0)
                for d in range(2):
                    yacc = yp.tile([P, TP, MSG], BF16, tag="yacc")
                    nc.vector.memset(yacc[:], 0.0)
                    for g in range(2):
                        ph = 2 * d + g
                        # boundary: load prebuilt table from DRAM stage
                        nc.sync.dma_start(out=tbl[:, :, 0], in_=tblst.ap()[:])
                        bstack = ExitStack()
                        if ph < 3:
                            btb = bstack.enter_context(
                                tc.tile_pool(name=f"tb{ph}", bufs=2))
                            btbx = bstack.enter_context(
                                tc.tile_pool(name=f"tbx{ph}", bufs=1))
                            btbps = bstack.enter_context(
                                tc.tile_pool(name=f"tbps{ph}", bufs=1,
                                             space="PSUM"))
                        bu = 0
                        # ---- gather + segment-sum for (d, g) ----
                        chunksl = chunks_all[d * 2 + g]
                        rows = rowsdg[d * 2 + g]
                        with tc.tile_pool(name="io", bufs=2) as iop, \
                             tc.tile_pool(name="go", bufs=1) as gop, \
                             tc.tile_pool(name="gb", bufs=1) as gbp, \
                             tc.tile_pool(name="oh", bufs=2) as ohp, \
                             tc.tile_pool(name="ts", bufs=2) as tsp, \
                             tc.tile_pool(name="mps", bufs=3,
                                          space="PSUM") as mps:
                            pos = 0
                            psy = None
                            for a0 in range(0, rows, NI):
                                ni = min(NI, rows - a0)
                                gi = iop.tile([P, NI // 16], I16, tag="gi")
                                nc.sync.dma_start(
                                    out=gi[:, :ni // 16],
                                    in_=gidx.ap()[:, g16ofs + a0 // 16:
                                                  g16ofs + (a0 + ni) // 16])
                                dlt = iop.tile([P, NI // P, UN, 1], BF16,
                                               tag="dlt")
                                nc.sync.dma_start(
                                    out=dlt[:, :ni // P, :, :],
                                    in_=dl8.ap()[:, ccofs + (a0 // P) * UN:
                                                 ccofs + ((a0 + ni) // P) * UN]
                                        .rearrange("p (c k one) -> p c k one",
                                                   k=UN, one=1))
                                go = gop.tile([P, NI, 1], F32, tag="go")
                                nc.gpsimd.ap_gather(
                                    go[:, :ni, :], tbl[:], gi[:, :ni // 16],
                                    channels=P, num_elems=NELEM, d=1,
                                    num_idxs=ni)
                                gb = gbp.tile([P, NI], BF16, tag="gbc")
                                nc.vector.tensor_copy(out=gb[:, :ni],
                                                      in_=go[:, :ni, 0])
                                for local in range(ni // P):
                                    t, r, first, last = chunksl[pos]
                                    pos += 1
                                    tps = mps.tile([P, P], BF16, space="PSUM",
                                                   tag="tps")
                                    nc.tensor.transpose(
                                        out=tps[:],
                                        in_=gb[:, local * P:(local + 1) * P],
                                        identity=ident16[:])
                                    tsb = tsp.tile([P, P], BF16, tag="tsb")
                                    nc.vector.tensor_copy(out=tsb[:], in_=tps[:])
                                    oh = ohp.tile([P, UN, P], BF16, tag="oh")
                                    nc.vector.tensor_tensor(
                                        out=oh[:],
                                        in0=dlt[:, local, :, :].to_broadcast(
                                            (P, UN, P)),
                                        in1=iota[:, 0:1, :].to_broadcast(
                                            (P, UN, P)),
                                        op=ALU.is_equal)
                                    if first:
                                        psy = mps.tile([P, 16], F32,
                                                       space="PSUM", tag="psy")
                                    for k in range(UN):
                                        nc.tensor.matmul(
                                            out=psy[:],
                                            lhsT=oh[:, k, :],
                                            rhs=tsb[:, 16 * k:16 * k + 16],
                                            start=(first and k == 0),
                                            stop=(last and k == UN - 1))
                                    if last:
                                        ys = yacc[:, t, r * 16:(r + 1) * 16]
                                        nc.vector.tensor_tensor(
                                            out=ys, in0=ys, in1=psy[:],
                                            op=ALU.add)
                                if ph < 3 and a0 > 0 and bu < UN:
                                    build_unit((ph + 1) // 2, (ph + 1) % 2,
                                               bu, btb, btbx, btbps)
                                    bu += 1
                        while ph < 3 and bu < UN:
                            build_unit((ph + 1) // 2, (ph + 1) % 2,
                                       bu, btb, btbx, btbps)
                            bu += 1
                        bstack.close()
                        g16ofs += rows // 16
                        ccofs += (rows // P) * UN
                    # ---------------- transform ----------------
                    with tc.tile_pool(name="p3", bufs=3) as p3, \
                         tc.tile_pool(name="p3ps", bufs=2, space="PSUM") as p3p:
                        for t in range(TP):
                            msg = p3.tile([P, MSG], F32, tag="msg")
                            nc.scalar.activation(
                                out=msg[:], in_=yacc[:, t, :],
                                func=ACTF.Relu,
                                scale=cisb[:, d * TP + t: d * TP + t + 1])
                            psmT = p3p.tile([MSG, P], F32, space="PSUM",
                                            tag="psmT")
                            nc.tensor.transpose(out=psmT[:], in_=msg[:],
                                                identity=ident[:])
                            msgT = p3.tile([MSG, P], F32, tag="msgT")
                            nc.vector.tensor_copy(out=msgT[:], in_=psmT[:])
                            fcp = p3p.tile([P, 64], F32, space="PSUM",
                                           tag="fcp")
                            nc.tensor.matmul(
                                out=fcp[:], lhsT=msgT[:], rhs=fcwT[:],
                                start=True, stop=True)
                            osb = p3.tile([P, 64], F32, tag="osb")
                            nc.vector.tensor_tensor(out=osb[:], in0=fcp[:],
                                                    in1=fcb[:], op=ALU.add)
                            dst = m_out if d == 0 else u_out
                            nc.sync.dma_start(
                                out=dst.ap()[t * P:(t + 1) * P], in_=osb[:])
    nc.compile()
    return nc


# ----------------------------------------------------------------- kernel

def make_in_maps(cfg, gidx, dl8, inputs):
    import ml_dtypes
    ins = {k: np.asarray(v) for k, v in inputs.items()}
    iota = np.tile(np.arange(P, dtype=ml_dtypes.bfloat16), (P, 1))
    dl8 = [d.astype(ml_dtypes.bfloat16) for d in dl8]
    base = dict(
        ufeat=ins['ufeat'], ifeat=ins['ifeat'],
        cj_u=ins['cj_user'], cj_m=ins['cj_movie'],
        attT=np.ascontiguousarray(ins['att'].T),
        basis2=ins['basis'].reshape(cfg.BAS, cfg.IN * 16).copy(),
        fc_w=ins['fc_w'], fc_b=ins['fc_b'].reshape(1, 64).copy(),
        iota128=iota,
    )
    in_maps = []
    for c in range(cfg.NCORES):
        ci = np.zeros((2 * cfg.TP * P, 1), np.float32)
        ci[:cfg.NSH] = ins['ci_movie'][c * cfg.NSH:(c + 1) * cfg.NSH]
        ci[cfg.TP * P:cfg.TP * P + cfg.NSH] = \
            ins['ci_user'][c * cfg.NSH:(c + 1) * cfg.NSH]
        in_maps.append({**base, 'ci_sh': ci, 'gidx': gidx[c], 'dl8': dl8[c]})
    return in_maps


def assemble(cfg, results):
    u = np.concatenate([results[c]['u_out'][:cfg.NSH]
                        for c in range(cfg.NCORES)])
    m = np.concatenate([results[c]['m_out'][:cfg.NSH]
                        for c in range(cfg.NCORES)])
    return u, m


def kernel(**inputs):
    from concourse import bass_utils
    cfg = Cfg()
    chunks_all, rowsdg, gidx, dl8 = build_plan(cfg, inputs['edge_user'],
                                               inputs['edge_movie'])
    nc = build_program(cfg, chunks_all, rowsdg, gidx[0].shape[1],
                       dl8[0].shape[1], cfg.NCORES)
    in_maps = make_in_maps(cfg, gidx, dl8, inputs)
    res = bass_utils.run_bass_kernel_spmd(nc, in_maps,
                                          core_ids=list(range(cfg.NCORES)))
    return assemble(cfg, res.results)


# revision 26
# speedup vs baseline: 1.1728x; 1.0173x over previous
"""GCMC layer on trn2 — v2: ap_gather (GPSIMD free-dim gather) + one-hot PE
segment-sum in transformed (16-lane) message space.

Design (per device, dst-sharded: device c owns dst nodes [c*NSH, (c+1)*NSH)):
  - Transform-first: x_r = (feat*cj) @ W_r  ([N,16] per rating) so each edge
    only moves 16 lanes. Table per (side, g): SBUF [128, 32000] f32 where
    partition 16k+j = lane j of slab (8g+k) (slab=6400 srcs), elem r*6400+s.
  - Edges binned per (d, g, unit k=src slab, dst-tile t, rating r); each
    (t, r) run padded to a 128-multiple of the max count over (device, unit)
    so the SPMD program is uniform. Unit streams are position-aligned: at any
    chunk all 8 units are in the same (t, r) run.
  - nc.gpsimd.ap_gather pulls per-edge x_r lanes from the SBUF table
    (per-unit int16 idx lists; Pool engine, ~6-9ns/idx/unit, 8 units in
    parallel — replaces dma_gather's ~8.6ns/row serial descriptor gen).
  - Per 128-row position: PE transpose -> T [128 e, 128 (k,j)]; DVE builds 8
    one-hots from dloc codes; 8 matmuls accumulate psum_y[dst,16] per run;
    DVE flushes into yacc [128, TP, 80] bf16.
  - Table build: stage fsrc16=(feat*cj) bf16 in DRAM; per slab DMA-transpose
    -> FT [64, 6400]; xT = Wall^T @ FT on PE; DVE copy psum->xTslab f32;
    5 HWDGE DMAs partition-remap xTslab -> table unit block.
  - Transform per (d, tile): relu(msg*ci) on ACT, PE transpose, f32 fc matmul
    + bias, dense store.
Host only bins/sorts indices and packs int16 idx + f32 dloc blocks.
"""
import sys
import numpy as np

sys.path.insert(0, '/opt/trn_rl_repo')

import concourse.bass as bass
import concourse.tile as tile
import concourse.mybir as mybir
from concourse import bacc
from concourse.masks import make_identity

F32 = mybir.dt.float32
BF16 = mybir.dt.bfloat16
I16 = mybir.dt.int16
ALU = mybir.AluOpType
ACTF = mybir.ActivationFunctionType
P = 128


class Cfg:
    def __init__(self, NU=100000, NM=100000, R=5, E=1000000, IN=64, BAS=4,
                 NCORES=8):
        assert NU == NM
        self.NU, self.NM, self.R, self.E, self.IN, self.BAS = NU, NM, R, E, IN, BAS
        self.MPR = 16
        self.MSG = self.MPR * R              # 80
        self.OUT = 64
        self.NCORES = NCORES
        self.NSH = NU // NCORES              # dsts per device
        self.TP = -(-self.NSH // P)          # dst tiles (98)
        self.UN = 8                          # gpsimd units
        self.SLAB = 6400                     # srcs per unit-slab
        self.NG = 2                          # slab groups (16 slabs total)
        self.NELEM = self.R * self.SLAB      # 32000 table elems per partition
        self.NUP = self.SLAB * self.UN * self.NG   # padded src count 102400
        self.NI = 5120                       # ap_gather rows per call


# ----------------------------------------------------------------- host prep

def build_plan(cfg, edge_user, edge_movie):
    """Bin edges per (d, g, unit, tile, rating); pad each (t, r) run to a
    128-multiple of the max count across (device, unit).

    Returns:
      chunks: per (d, g): list of (t, r, first, last) per 128-row chunk
      rowsdg: per (d, g): padded rows per unit
      gidx:  per-device int16 [128, G16] idx blocks (concat over d, g)
      dl8:   per-device f32  [128, CC*8] dloc blocks (-1 = padding)
    """
    NC, UN, TP, R = cfg.NCORES, cfg.UN, cfg.TP, cfg.R
    NSH, SLAB = cfg.NSH, cfg.SLAB
    eu = np.asarray(edge_user)
    em = np.asarray(edge_movie)

    chunks_all = []
    rowsdg = []
    gparts = [[] for _ in range(NC)]
    dparts = [[] for _ in range(NC)]
    for d in range(2):
        src_all, dst_all = (eu, em) if d == 0 else (em, eu)
        src = src_all.reshape(-1).astype(np.int64)
        dst = dst_all.reshape(-1).astype(np.int64)
        rr = np.repeat(np.arange(R, dtype=np.int64), cfg.E)
        c = dst // NSH
        ld = dst % NSH
        t = ld // P
        dl = (ld % P).astype(np.float32)
        slab = src // SLAB
        g = slab // UN
        k = slab % UN
        s = src % SLAB
        idx = (rr * SLAB + s).astype(np.int16)
        for gv in range(2):
            m = g == gv
            key = ((c[m] * UN + k[m]) * TP + t[m]) * R + rr[m]
            cnt = np.bincount(key, minlength=NC * UN * TP * R)
            cnt = cnt.reshape(NC, UN, TP, R)
            nch = -(-cnt.max(axis=(0, 1)) // P)          # [TP, R] chunks
            L = nch * P
            Lf = L.reshape(-1)
            base = np.concatenate([[0], np.cumsum(Lf)[:-1]]).reshape(TP, R)
            rows = int(Lf.sum())
            rowsdg.append(rows)
            # rank within (c,k,t,r)
            order = np.argsort(key, kind='stable')
            ko = key[order]
            bnd = np.flatnonzero(np.diff(ko, prepend=-1))
            rank = np.arange(ko.size) - np.repeat(
                bnd, np.diff(np.append(bnd, ko.size)))
            inv = np.empty_like(order)
            inv[order] = np.arange(order.size)
            rank = rank[inv]
            pos = base[t[m], rr[m]] + rank
            gs = np.zeros((NC, UN, rows), np.int16)
            dv = np.full((NC, UN, rows), -1.0, np.float32)
            gs[c[m], k[m], pos] = idx[m]
            dv[c[m], k[m], pos] = dl[m]
            for cc in range(NC):
                gb = np.zeros((P, rows // 16), np.int16)
                db = np.zeros((P, (rows // P) * UN), np.float32)
                dbv = db.reshape(P, rows // P, UN)
                for kk in range(UN):
                    gb[16 * kk:16 * kk + 16] = \
                        gs[cc, kk].reshape(-1, 16).T
                    dbv[:, :, kk] = dv[cc, kk].reshape(-1, P).T
                gparts[cc].append(gb)
                dparts[cc].append(db)
            ch = []
            for tt in range(TP):
                for r in range(R):
                    n = int(nch[tt, r])
                    for j in range(n):
                        ch.append((tt, r, j == 0, j == n - 1))
            chunks_all.append(ch)
    gidx = [np.concatenate(gp, axis=1) for gp in gparts]
    dl8 = [np.concatenate(dp, axis=1) for dp in dparts]
    return chunks_all, rowsdg, gidx, dl8


# ------------------------------------------------------------- numpy model

def model(cfg, chunks_all, rowsdg, gidx, dl8, inputs):
    """Numpy mirror of the device program (f32, no bf16 rounding)."""
    import ml_dtypes
    BF = ml_dtypes.bfloat16
    R, NSH, TP, SLAB, UN = cfg.R, cfg.NSH, cfg.TP, cfg.SLAB, cfg.UN
    W = np.einsum('rb,bio->rio', np.asarray(inputs['att']),
                  np.asarray(inputs['basis'])).astype(np.float32)
    fc_w = np.asarray(inputs['fc_w'])
    fc_b = np.asarray(inputs['fc_b'])
    xfull = np.zeros((2, cfg.NUP, cfg.MSG), np.float32)
    for side, (f, cj) in enumerate((('ufeat', 'cj_user'), ('ifeat', 'cj_movie'))):
        fc = (np.asarray(inputs[f]) * np.asarray(inputs[cj])).astype(BF)
        for r in range(R):
            xfull[side, :cfg.NU, 16 * r:16 * r + 16] = \
                fc.astype(np.float32) @ W[r].astype(BF).astype(np.float32)
    ci = [np.asarray(inputs['ci_movie']), np.asarray(inputs['ci_user'])]

    u_out = np.zeros((cfg.NU, cfg.OUT), np.float32)
    m_out = np.zeros((cfg.NM, cfg.OUT), np.float32)
    for c in range(cfg.NCORES):
        g16o = 0
        cco = 0
        for d in range(2):
            yacc = np.zeros((TP * P, cfg.MSG), np.float32)
            for g in range(2):
                ch = chunks_all[d * 2 + g]
                rows = rowsdg[d * 2 + g]
                gb = gidx[c][:, g16o:g16o + rows // 16]
                db = dl8[c][:, cco:cco + (rows // P) * UN].reshape(
                    P, rows // P, UN)
                g16o += rows // 16
                cco += (rows // P) * UN
                trow = np.repeat([t for (t, r, _, _) in ch], P)
                rrow = np.repeat([r for (t, r, _, _) in ch], P)
                for k in range(UN):
                    idxs = gb[16 * k:16 * k + 16].T.reshape(-1)  # [rows]
                    dls = db[:, :, k].T.reshape(-1)
                    msk = dls >= 0
                    base = (8 * g + k) * SLAB
                    elem = idxs.astype(np.int64)
                    s = elem % SLAB
                    rv = elem // SLAB
                    val = np.zeros((rows, 16), np.float32)
                    sel = xfull[d, base + s]                      # [rows, 80]
                    val = sel[np.arange(rows)[:, None],
                              (rv * 16)[:, None] + np.arange(16)[None, :]]
                    tgt = trow * P + dls.astype(np.int64)
                    col = rrow * 16
                    np.add.at(yacc, (tgt[msk][:, None],
                                     (col[msk][:, None] + np.arange(16))),
                              val[msk])
            cish = np.zeros((TP * P, 1), np.float32)
            cish[:NSH] = ci[d][c * NSH:(c + 1) * NSH]
            z = np.maximum(yacc * cish, 0.0) @ fc_w.T + fc_b
            if d == 0:
                m_out[c * NSH:(c + 1) * NSH] = z[:NSH]
            else:
                u_out[c * NSH:(c + 1) * NSH] = z[:NSH]
    return u_out, m_out


# ---------------------------------------------------------- device program

def build_program(cfg, chunks_all, rowsdg, g16cols, cccols, num_devices):
    nc = bacc.Bacc("TRN2", target_bir_lowering=False, debug=False,
                   num_devices=num_devices)
    NU, IN, R, BAS = cfg.NU, cfg.IN, cfg.R, cfg.BAS
    TP, UN, SLAB, NELEM, MSG, NI = (cfg.TP, cfg.UN, cfg.SLAB, cfg.NELEM,
                                    cfg.MSG, cfg.NI)

    ufeat = nc.dram_tensor("ufeat", (NU, IN), F32, kind="ExternalInput")
    ifeat = nc.dram_tensor("ifeat", (NU, IN), F32, kind="ExternalInput")
    cj_u = nc.dram_tensor("cj_u", (NU, 1), F32, kind="ExternalInput")
    cj_m = nc.dram_tensor("cj_m", (NU, 1), F32, kind="ExternalInput")
    ci_sh = nc.dram_tensor("ci_sh", (2 * TP * P, 1), F32, kind="ExternalInput")
    attT = nc.dram_tensor("attT", (BAS, R), F32, kind="ExternalInput")
    basis2 = nc.dram_tensor("basis2", (BAS, IN * 16), F32, kind="ExternalInput")
    fc_w = nc.dram_tensor("fc_w", (64, MSG), F32, kind="ExternalInput")
    fc_b = nc.dram_tensor("fc_b", (1, 64), F32, kind="ExternalInput")
    iota_d = nc.dram_tensor("iota128", (P, P), BF16, kind="ExternalInput")
    gidx = nc.dram_tensor("gidx", (P, g16cols), I16, kind="ExternalInput")
    dl8 = nc.dram_tensor("dl8", (P, cccols), BF16, kind="ExternalInput")

    m_out = nc.dram_tensor("m_out", (TP * P, 64), F32, kind="ExternalOutput")
    u_out = nc.dram_tensor("u_out", (TP * P, 64), F32, kind="ExternalOutput")

    wscr = nc.dram_tensor("wscr", (R, IN * 16), F32, kind="Internal")
    fsrc16 = nc.dram_tensor("fsrc16", (2 * cfg.NUP, IN), BF16, kind="Internal")
    tblst = nc.dram_tensor("tblst", (P, NELEM), F32, kind="Internal")

    with tile.TileContext(nc) as tc:
        with tc.tile_pool(name="const", bufs=1) as pool:
            # ---------------- constants ----------------
            pp_ctx = tc.tile_pool(name="cpsum", bufs=2, space="PSUM")
            pp = pp_ctx.__enter__()
            ident = pool.tile([P, P], F32)
            make_identity(nc, ident[:])
            ident16 = pool.tile([P, P], BF16)
            make_identity(nc, ident16[:])

            with tc.tile_pool(name="w0", bufs=1) as wp:
                at = wp.tile([BAS, R], F32)
                bs = wp.tile([BAS, IN * 16], F32)
                nc.sync.dma_start(out=at[:], in_=attT.ap()[:])
                nc.sync.dma_start(out=bs[:], in_=basis2.ap()[:])
                w5 = wp.tile([R, IN * 16], F32)
                half = IN * 16 // 2
                for h in range(2):
                    ps = pp.tile([R, half], F32, space="PSUM", tag="w5ps")
                    nc.tensor.matmul(out=ps[:], lhsT=at[:],
                                     rhs=bs[:, h * half:(h + 1) * half],
                                     start=True, stop=True)
                    nc.scalar.copy(out=w5[:, h * half:(h + 1) * half], in_=ps[:])
                nc.sync.dma_start(out=wscr.ap()[:], in_=w5[:])
            w64 = pool.tile([IN, R, 16], F32)
            nc.sync.dma_start(
                out=w64[:], in_=wscr.ap()[:].rearrange("r (k o) -> k r o", k=IN))
            wallb = pool.tile([IN, R * 16], BF16)
            nc.scalar.copy(out=wallb[:],
                           in_=w64[:].rearrange("k r o -> k (r o)"))

            fcw = pool.tile([64, MSG], F32)
            nc.sync.dma_start(out=fcw[:], in_=fc_w.ap()[:])
            psT = pp.tile([MSG, 64], F32, space="PSUM", tag="fcT")
            nc.tensor.transpose(out=psT[:], in_=fcw[:], identity=ident[:64, :64])
            fcwT = pool.tile([MSG, 64], F32)
            nc.scalar.copy(out=fcwT[:], in_=psT[:])
            fcb = pool.tile([P, 64], F32)
            nc.sync.dma_start(out=fcb[:], in_=fc_b.ap()[:].to_broadcast((P, 64)))

            cisb = pool.tile([P, 2 * TP], F32)
            nc.sync.dma_start(
                out=cisb[:],
                in_=ci_sh.ap()[:].rearrange("(t p) o -> p (t o)", p=P))
            iota = pool.tile([P, 1, P], BF16)
            nc.sync.dma_start(out=iota[:, 0, :], in_=iota_d.ap()[:])
            pp_ctx.__exit__(None, None, None)

            # ---------------- stage fsrc16 = (feat*cj) bf16 ----------------
            # Side 0 staged up front; side 1 staged during d=0's first
            # gather phase so its DMAs/DVE hide under the Pool gathers.
            GT = 8

            def stage_side(side, feat, cj):
                with tc.tile_pool(name=f"p1{side}", bufs=3) as p1:
                    starts = list(range(0, NU - GT * P + 1, GT * P))
                    if NU % (GT * P):
                        starts.append(NU - GT * P)
                    for g0 in starts:
                        ft = p1.tile([P, GT, IN], F32, tag="ft")
                        cjt = p1.tile([P, GT, 1], F32, tag="cj")
                        nc.sync.dma_start(
                            out=ft[:], in_=feat.ap()[g0:g0 + GT * P].rearrange(
                                "(p a) d -> p a d", p=P))
                        nc.sync.dma_start(
                            out=cjt[:], in_=cj.ap()[g0:g0 + GT * P].rearrange(
                                "(p a) d -> p a d", p=P))
                        sc = p1.tile([P, GT, IN], BF16, tag="sc")
                        nc.vector.tensor_tensor(
                            out=sc[:], in0=ft[:],
                            in1=cjt[:].to_broadcast((P, GT, IN)),
                            op=ALU.mult)
                        ofs = side * cfg.NUP + g0
                        nc.sync.dma_start(
                            out=fsrc16.ap()[ofs:ofs + GT * P]
                                .rearrange("(p a) d -> p a d", p=P),
                            in_=sc[:])

            stage_side(0, ufeat, cj_u)

            # Build one unit-slab of the (dd, gg) table into the DRAM stage.
            # Emitted interleaved between gather calls of the previous phase
            # so PE/DVE slices fit in per-call slack instead of front-loading
            # the engine FIFOs.
            def build_unit(dd, gg, k, tb, tbx, tbps):
                base = dd * cfg.NUP + (UN * gg + k) * SLAB
                for hh in range(2):
                    half = SLAB // 2
                    xts = tbx.tile([MSG, half], F32, tag="xts")
                    for c0 in range(0, half, 512):
                        w = min(512, half - c0)
                        fc4 = tb.tile([P, 4, IN], BF16, tag="fc4")
                        nc.sync.dma_start(
                            out=fc4[:, :w // P, :],
                            in_=fsrc16.ap()[
                                base + hh * half + c0:
                                base + hh * half + c0 + w]
                                .rearrange("(a p) d -> p a d", p=P))
                        ft4 = tb.tile([IN, 4, P], BF16, tag="ft4")
                        for j in range(w // P):
                            fps = tbps.tile([IN, P], BF16, space="PSUM",
                                            tag="fps")
                            nc.tensor.transpose(
                                out=fps[:], in_=fc4[:, j, :],
                                identity=ident16[:])
                            nc.vector.tensor_copy(
                                out=ft4[:, j, :], in_=fps[:])
                        xps = tbps.tile([MSG, 512], F32, space="PSUM",
                                        tag="xps")
                        nc.tensor.matmul(
                            out=xps[:, :w], lhsT=wallb[:],
                            rhs=ft4[:].rearrange("f a p -> f (a p)")[:, :w],
                            start=True, stop=True)
                        nc.vector.tensor_copy(
                            out=xts[:, c0:c0 + w], in_=xps[:, :w])
                    for r in range(R):
                        nc.sync.dma_start(
                            out=tblst.ap()[
                                16 * k:16 * k + 16,
                                r * SLAB + hh * (SLAB // 2):
                                r * SLAB + (hh + 1) * (SLAB // 2)],
                            in_=xts[16 * r:16 * r + 16, :])

            # ---------------- main: per direction ----------------
            g16ofs = 0
            ccofs = 0
            with tc.tile_pool(name="tblp", bufs=1) as tblp, \
                 tc.tile_pool(name="yaccp", bufs=1) as yp:
                tbl = tblp.tile([P, NELEM, 1], F32)
                from contextlib import ExitStack
                with tc.tile_pool(name="tbi", bufs=2) as tb0, \
                     tc.tile_pool(name="tbxi", bufs=1) as tbx0, \
                     tc.tile_pool(name="tbpsi", bufs=2, space="PSUM") as tbps0:
                    for k in range(UN):
                        build_unit(0, 0, k, tb0, tbx0, tbps0)
                nc.sync.dma_start(out=tbl[:, :, 0], in_=tblst.ap()[:])
                stage_side(1, ifeat, cj_m)
                for d in range(2):
                    yacc = yp.tile([P, TP, MSG], BF16, tag="yacc")
                    nc.vector.memset(yacc[:], 0.0)
                    for g in range(2):
                        ph = 2 * d + g
                        bstack = ExitStack()
                        if ph < 3:
                            btb = bstack.enter_context(
                                tc.tile_pool(name=f"tb{ph}", bufs=2))
                            btbx = bstack.enter_context(
                                tc.tile_pool(name=f"tbx{ph}", bufs=1))
                            btbps = bstack.enter_context(
                                tc.tile_pool(name=f"tbps{ph}", bufs=1,
                                             space="PSUM"))
                        bu = 0
                        # ---- gather + segment-sum for (d, g) ----
                        chunksl = chunks_all[d * 2 + g]
                        rows = rowsdg[d * 2 + g]
                        with tc.tile_pool(name="io", bufs=2) as iop, \
                             tc.tile_pool(name="go", bufs=1) as gop, \
                             tc.tile_pool(name="gb", bufs=1) as gbp, \
                             tc.tile_pool(name="oh", bufs=2) as ohp, \
                             tc.tile_pool(name="ts", bufs=2) as tsp, \
                             tc.tile_pool(name="mps", bufs=3,
                                          space="PSUM") as mps:
                            pos = 0
                            psy = None
                            for a0 in range(0, rows, NI):
                                ni = min(NI, rows - a0)
                                gi = iop.tile([P, NI // 16], I16, tag="gi")
                                nc.sync.dma_start(
                                    out=gi[:, :ni // 16],
                                    in_=gidx.ap()[:, g16ofs + a0 // 16:
                                                  g16ofs + (a0 + ni) // 16])
                                dlt = iop.tile([P, NI // P, UN, 1], BF16,
                                               tag="dlt")
                                nc.sync.dma_start(
                                    out=dlt[:, :ni // P, :, :],
                                    in_=dl8.ap()[:, ccofs + (a0 // P) * UN:
                                                 ccofs + ((a0 + ni) // P) * UN]
                                        .rearrange("p (c k one) -> p c k one",
                                                   k=UN, one=1))
                                go = gop.tile([P, NI, 1], F32, tag="go")
                                nc.gpsimd.ap_gather(
                                    go[:, :ni, :], tbl[:], gi[:, :ni // 16],
                                    channels=P, num_elems=NELEM, d=1,
                                    num_idxs=ni)
                                gb = gbp.tile([P, NI], BF16, tag="gbc")
                                nc.vector.tensor_copy(out=gb[:, :ni],
                                                      in_=go[:, :ni, 0])
                                for local in range(ni // P):
                                    t, r, first, last = chunksl[pos]
                                    pos += 1
                                    tps = mps.tile([P, P], BF16, space="PSUM",
                                                   tag="tps")
                                    nc.tensor.transpose(
                                        out=tps[:],
                                        in_=gb[:, local * P:(local + 1) * P],
                                        identity=ident16[:])
                                    tsb = tsp.tile([P, P], BF16, tag="tsb")
                                    nc.vector.tensor_copy(out=tsb[:], in_=tps[:])
                                    oh = ohp.tile([P, UN, P], BF16, tag="oh")
                                    nc.vector.tensor_tensor(
                                        out=oh[:],
                                        in0=dlt[:, local, :, :].to_broadcast(
                                            (P, UN, P)),
                                        in1=iota[:, 0:1, :].to_broadcast(
                                            (P, UN, P)),
                                        op=ALU.is_equal)
                                    if first:
                                        psy = mps.tile([P, 16], F32,
                                                       space="PSUM", tag="psy")
                                    for k in range(UN):
                                        nc.tensor.matmul(
                                            out=psy[:],
                                            lhsT=oh[:, k, :],
                                            rhs=tsb[:, 16 * k:16 * k + 16],
                                            start=(first and k == 0),
                                            stop=(last and k == UN - 1))
                                    if last:
                                        ys = yacc[:, t, r * 16:(r + 1) * 16]
                                        nc.vector.tensor_tensor(
                                            out=ys, in0=ys, in1=psy[:],
                                            op=ALU.add)
                                if ph < 3 and a0 > 0 and bu < UN:
                                    build_unit((ph + 1) // 2, (ph + 1) % 2,
                                               bu, btb, btbx, btbps)
                                    bu += 1
                        while ph < 3 and bu < UN:
                            build_unit((ph + 1) // 2, (ph + 1) % 2,
                                       bu, btb, btbx, btbps)
                            bu += 1
                        if ph < 3:
                            # load next phase's table now, ahead of the
                            # transform DMAs in the sync-engine FIFO
                            nc.sync.dma_start(out=tbl[:, :, 0],
                                              in_=tblst.ap()[:])
                        bstack.close()
                        g16ofs += rows // 16
                        ccofs += (rows // P) * UN
                    # ---------------- transform ----------------
                    with tc.tile_pool(name="p3", bufs=3) as p3, \
                         tc.tile_pool(name="p3ps", bufs=2, space="PSUM") as p3p:
                        for t in range(TP):
                            msg = p3.tile([P, MSG], F32, tag="msg")
                            nc.scalar.activation(
                                out=msg[:], in_=yacc[:, t, :],
                                func=ACTF.Relu,
                                scale=cisb[:, d * TP + t: d * TP + t + 1])
                            psmT = p3p.tile([MSG, P], F32, space="PSUM",
                                            tag="psmT")
                            nc.tensor.transpose(out=psmT[:], in_=msg[:],
                                                identity=ident[:])
                            msgT = p3.tile([MSG, P], F32, tag="msgT")
                            nc.vector.tensor_copy(out=msgT[:], in_=psmT[:])
                            fcp = p3p.tile([P, 64], F32, space="PSUM",
                                           tag="fcp")
                            nc.tensor.matmul(
                                out=fcp[:], lhsT=msgT[:], rhs=fcwT[:],
                                start=True, stop=True)
                            osb = p3.tile([P, 64], F32, tag="osb")
                            nc.vector.tensor_tensor(out=osb[:], in0=fcp[:],
                                                    in1=fcb[:], op=ALU.add)
                            dst = m_out if d == 0 else u_out
                            nc.sync.dma_start(
                                out=dst.ap()[t * P:(t + 1) * P], in_=osb[:])
    nc.compile()
    return nc


# ----------------------------------------------------------------- kernel

def make_in_maps(cfg, gidx, dl8, inputs):
    import ml_dtypes
    ins = {k: np.asarray(v) for k, v in inputs.items()}
    iota = np.tile(np.arange(P, dtype=ml_dtypes.bfloat16), (P, 1))
    dl8 = [d.astype(ml_dtypes.bfloat16) for d in dl8]
    base = dict(
        ufeat=ins['ufeat'], ifeat=ins['ifeat'],
        cj_u=ins['cj_user'], cj_m=ins['cj_movie'],
        attT=np.ascontiguousarray(ins['att'].T),
        basis2=ins['basis'].reshape(cfg.BAS, cfg.IN * 16).copy(),
        fc_w=ins['fc_w'], fc_b=ins['fc_b'].reshape(1, 64).copy(),
        iota128=iota,
    )
    in_maps = []
    for c in range(cfg.NCORES):
        ci = np.zeros((2 * cfg.TP * P, 1), np.float32)
        ci[:cfg.NSH] = ins['ci_movie'][c * cfg.NSH:(c + 1) * cfg.NSH]
        ci[cfg.TP * P:cfg.TP * P + cfg.NSH] = \
            ins['ci_user'][c * cfg.NSH:(c + 1) * cfg.NSH]
        in_maps.append({**base, 'ci_sh': ci, 'gidx': gidx[c], 'dl8': dl8[c]})
    return in_maps


def assemble(cfg, results):
    u = np.concatenate([results[c]['u_out'][:cfg.NSH]
                        for c in range(cfg.NCORES)])
    m = np.concatenate([results[c]['m_out'][:cfg.NSH]
                        for c in range(cfg.NCORES)])
    return u, m


def kernel(**inputs):
    from concourse import bass_utils
    cfg = Cfg()
    chunks_all, rowsdg, gidx, dl8 = build_plan(cfg, inputs['edge_user'],
                                               inputs['edge_movie'])
    nc = build_program(cfg, chunks_all, rowsdg, gidx[0].shape[1],
                       dl8[0].shape[1], cfg.NCORES)
    in_maps = make_in_maps(cfg, gidx, dl8, inputs)
    res = bass_utils.run_bass_kernel_spmd(nc, in_maps,
                                          core_ids=list(range(cfg.NCORES)))
    return assemble(cfg, res.results)


# revision 28
# speedup vs baseline: 1.1738x; 1.0008x over previous
"""GCMC layer on trn2 — v2: ap_gather (GPSIMD free-dim gather) + one-hot PE
segment-sum in transformed (16-lane) message space.

Design (per device, dst-sharded: device c owns dst nodes [c*NSH, (c+1)*NSH)):
  - Transform-first: x_r = (feat*cj) @ W_r  ([N,16] per rating) so each edge
    only moves 16 lanes. Table per (side, g): SBUF [128, 32000] f32 where
    partition 16k+j = lane j of slab (8g+k) (slab=6400 srcs), elem r*6400+s.
  - Edges binned per (d, g, unit k=src slab, dst-tile t, rating r); each
    (t, r) run padded to a 128-multiple of the max count over (device, unit)
    so the SPMD program is uniform. Unit streams are position-aligned: at any
    chunk all 8 units are in the same (t, r) run.
  - nc.gpsimd.ap_gather pulls per-edge x_r lanes from the SBUF table
    (per-unit int16 idx lists; Pool engine, ~6-9ns/idx/unit, 8 units in
    parallel — replaces dma_gather's ~8.6ns/row serial descriptor gen).
  - Per 128-row position: PE transpose -> T [128 e, 128 (k,j)]; DVE builds 8
    one-hots from dloc codes; 8 matmuls accumulate psum_y[dst,16] per run;
    DVE flushes into yacc [128, TP, 80] bf16.
  - Table build: stage fsrc16=(feat*cj) bf16 in DRAM; per slab DMA-transpose
    -> FT [64, 6400]; xT = Wall^T @ FT on PE; DVE copy psum->xTslab f32;
    5 HWDGE DMAs partition-remap xTslab -> table unit block.
  - Transform per (d, tile): relu(msg*ci) on ACT, PE transpose, f32 fc matmul
    + bias, dense store.
Host only bins/sorts indices and packs int16 idx + f32 dloc blocks.
"""
import sys
import numpy as np

sys.path.insert(0, '/opt/trn_rl_repo')

import concourse.bass as bass
import concourse.tile as tile
import concourse.mybir as mybir
from concourse import bacc
from concourse.masks import make_identity

F32 = mybir.dt.float32
BF16 = mybir.dt.bfloat16
I16 = mybir.dt.int16
ALU = mybir.AluOpType
ACTF = mybir.ActivationFunctionType
P = 128


class Cfg:
    def __init__(self, NU=100000, NM=100000, R=5, E=1000000, IN=64, BAS=4,
                 NCORES=8):
        assert NU == NM
        self.NU, self.NM, self.R, self.E, self.IN, self.BAS = NU, NM, R, E, IN, BAS
        self.MPR = 16
        self.MSG = self.MPR * R              # 80
        self.OUT = 64
        self.NCORES = NCORES
        self.NSH = NU // NCORES              # dsts per device
        self.TP = -(-self.NSH // P)          # dst tiles (98)
        self.UN = 8                          # gpsimd units
        self.SLAB = 6400                     # srcs per unit-slab
        self.NG = 2                          # slab groups (16 slabs total)
        self.NELEM = self.R * self.SLAB      # 32000 table elems per partition
        self.NUP = self.SLAB * self.UN * self.NG   # padded src count 102400
        self.NI = 5120                       # ap_gather rows per call


# ----------------------------------------------------------------- host prep

def build_plan(cfg, edge_user, edge_movie):
    """Bin edges per (d, g, unit, tile, rating); pad each (t, r) run to a
    128-multiple of the max count across (device, unit).

    Returns:
      chunks: per (d, g): list of (t, r, first, last) per 128-row chunk
      rowsdg: per (d, g): padded rows per unit
      gidx:  per-device int16 [128, G16] idx blocks (concat over d, g)
      dl8:   per-device f32  [128, CC*8] dloc blocks (-1 = padding)
    """
    NC, UN, TP, R = cfg.NCORES, cfg.UN, cfg.TP, cfg.R
    NSH, SLAB = cfg.NSH, cfg.SLAB
    eu = np.asarray(edge_user)
    em = np.asarray(edge_movie)

    chunks_all = []
    rowsdg = []
    gparts = [[] for _ in range(NC)]
    dparts = [[] for _ in range(NC)]
    for d in range(2):
        src_all, dst_all = (eu, em) if d == 0 else (em, eu)
        src = src_all.reshape(-1).astype(np.int64)
        dst = dst_all.reshape(-1).astype(np.int64)
        rr = np.repeat(np.arange(R, dtype=np.int64), cfg.E)
        c = dst // NSH
        ld = dst % NSH
        t = ld // P
        dl = (ld % P).astype(np.float32)
        slab = src // SLAB
        g = slab // UN
        k = slab % UN
        s = src % SLAB
        idx = (rr * SLAB + s).astype(np.int16)
        for gv in range(2):
            m = g == gv
            key = ((c[m] * UN + k[m]) * TP + t[m]) * R + rr[m]
            cnt = np.bincount(key, minlength=NC * UN * TP * R)
            cnt = cnt.reshape(NC, UN, TP, R)
            nch = -(-cnt.max(axis=(0, 1)) // P)          # [TP, R] chunks
            L = nch * P
            Lf = L.reshape(-1)
            base = np.concatenate([[0], np.cumsum(Lf)[:-1]]).reshape(TP, R)
            rows = int(Lf.sum())
            rowsdg.append(rows)
            # rank within (c,k,t,r)
            order = np.argsort(key, kind='stable')
            ko = key[order]
            bnd = np.flatnonzero(np.diff(ko, prepend=-1))
            rank = np.arange(ko.size) - np.repeat(
                bnd, np.diff(np.append(bnd, ko.size)))
            inv = np.empty_like(order)
            inv[order] = np.arange(order.size)
            rank = rank[inv]
            pos = base[t[m], rr[m]] + rank
            gs = np.zeros((NC, UN, rows), np.int16)
            dv = np.full((NC, UN, rows), -1.0, np.float32)
            gs[c[m], k[m], pos] = idx[m]
            dv[c[m], k[m], pos] = dl[m]
            for cc in range(NC):
                gb = np.zeros((P, rows // 16), np.int16)
                db = np.zeros((P, (rows // P) * UN), np.float32)
                dbv = db.reshape(P, rows // P, UN)
                for kk in range(UN):
                    gb[16 * kk:16 * kk + 16] = \
                        gs[cc, kk].reshape(-1, 16).T
                    dbv[:, :, kk] = dv[cc, kk].reshape(-1, P).T
                gparts[cc].append(gb)
                dparts[cc].append(db)
            ch = []
            for tt in range(TP):
                for r in range(R):
                    n = int(nch[tt, r])
                    for j in range(n):
                        ch.append((tt, r, j == 0, j == n - 1))
            chunks_all.append(ch)
    gidx = [np.concatenate(gp, axis=1) for gp in gparts]
    dl8 = [np.concatenate(dp, axis=1) for dp in dparts]
    return chunks_all, rowsdg, gidx, dl8


# ------------------------------------------------------------- numpy model

def model(cfg, chunks_all, rowsdg, gidx, dl8, inputs):
    """Numpy mirror of the device program (f32, no bf16 rounding)."""
    import ml_dtypes
    BF = ml_dtypes.bfloat16
    R, NSH, TP, SLAB, UN = cfg.R, cfg.NSH, cfg.TP, cfg.SLAB, cfg.UN
    W = np.einsum('rb,bio->rio', np.asarray(inputs['att']),
                  np.asarray(inputs['basis'])).astype(np.float32)
    fc_w = np.asarray(inputs['fc_w'])
    fc_b = np.asarray(inputs['fc_b'])
    xfull = np.zeros((2, cfg.NUP, cfg.MSG), np.float32)
    for side, (f, cj) in enumerate((('ufeat', 'cj_user'), ('ifeat', 'cj_movie'))):
        fc = (np.asarray(inputs[f]) * np.asarray(inputs[cj])).astype(BF)
        for r in range(R):
            xfull[side, :cfg.NU, 16 * r:16 * r + 16] = \
                fc.astype(np.float32) @ W[r].astype(BF).astype(np.float32)
    ci = [np.asarray(inputs['ci_movie']), np.asarray(inputs['ci_user'])]

    u_out = np.zeros((cfg.NU, cfg.OUT), np.float32)
    m_out = np.zeros((cfg.NM, cfg.OUT), np.float32)
    for c in range(cfg.NCORES):
        g16o = 0
        cco = 0
        for d in range(2):
            yacc = np.zeros((TP * P, cfg.MSG), np.float32)
            for g in range(2):
                ch = chunks_all[d * 2 + g]
                rows = rowsdg[d * 2 + g]
                gb = gidx[c][:, g16o:g16o + rows // 16]
                db = dl8[c][:, cco:cco + (rows // P) * UN].reshape(
                    P, rows // P, UN)
                g16o += rows // 16
                cco += (rows // P) * UN
                trow = np.repeat([t for (t, r, _, _) in ch], P)
                rrow = np.repeat([r for (t, r, _, _) in ch], P)
                for k in range(UN):
                    idxs = gb[16 * k:16 * k + 16].T.reshape(-1)  # [rows]
                    dls = db[:, :, k].T.reshape(-1)
                    msk = dls >= 0
                    base = (8 * g + k) * SLAB
                    elem = idxs.astype(np.int64)
                    s = elem % SLAB
                    rv = elem // SLAB
                    val = np.zeros((rows, 16), np.float32)
                    sel = xfull[d, base + s]                      # [rows, 80]
                    val = sel[np.arange(rows)[:, None],
                              (rv * 16)[:, None] + np.arange(16)[None, :]]
                    tgt = trow * P + dls.astype(np.int64)
                    col = rrow * 16
                    np.add.at(yacc, (tgt[msk][:, None],
                                     (col[msk][:, None] + np.arange(16))),
                              val[msk])
            cish = np.zeros((TP * P, 1), np.float32)
            cish[:NSH] = ci[d][c * NSH:(c + 1) * NSH]
            z = np.maximum(yacc * cish, 0.0) @ fc_w.T + fc_b
            if d == 0:
                m_out[c * NSH:(c + 1) * NSH] = z[:NSH]
            else:
                u_out[c * NSH:(c + 1) * NSH] = z[:NSH]
    return u_out, m_out


# ---------------------------------------------------------- device program

def build_program(cfg, chunks_all, rowsdg, g16cols, cccols, num_devices):
    nc = bacc.Bacc("TRN2", target_bir_lowering=False, debug=False,
                   num_devices=num_devices)
    NU, IN, R, BAS = cfg.NU, cfg.IN, cfg.R, cfg.BAS
    TP, UN, SLAB, NELEM, MSG, NI = (cfg.TP, cfg.UN, cfg.SLAB, cfg.NELEM,
                                    cfg.MSG, cfg.NI)

    ufeat = nc.dram_tensor("ufeat", (NU, IN), F32, kind="ExternalInput")
    ifeat = nc.dram_tensor("ifeat", (NU, IN), F32, kind="ExternalInput")
    cj_u = nc.dram_tensor("cj_u", (NU, 1), F32, kind="ExternalInput")
    cj_m = nc.dram_tensor("cj_m", (NU, 1), F32, kind="ExternalInput")
    ci_sh = nc.dram_tensor("ci_sh", (2 * TP * P, 1), F32, kind="ExternalInput")
    attT = nc.dram_tensor("attT", (BAS, R), F32, kind="ExternalInput")
    basis2 = nc.dram_tensor("basis2", (BAS, IN * 16), F32, kind="ExternalInput")
    fc_w = nc.dram_tensor("fc_w", (64, MSG), F32, kind="ExternalInput")
    fc_b = nc.dram_tensor("fc_b", (1, 64), F32, kind="ExternalInput")
    iota_d = nc.dram_tensor("iota128", (P, P), BF16, kind="ExternalInput")
    gidx = nc.dram_tensor("gidx", (P, g16cols), I16, kind="ExternalInput")
    dl8 = nc.dram_tensor("dl8", (P, cccols), BF16, kind="ExternalInput")

    m_out = nc.dram_tensor("m_out", (TP * P, 64), F32, kind="ExternalOutput")
    u_out = nc.dram_tensor("u_out", (TP * P, 64), F32, kind="ExternalOutput")

    wscr = nc.dram_tensor("wscr", (R, IN * 16), F32, kind="Internal")
    fsrc16 = nc.dram_tensor("fsrc16", (2 * cfg.NUP, IN), BF16, kind="Internal")
    tblst = nc.dram_tensor("tblst", (P, NELEM), F32, kind="Internal")

    with tile.TileContext(nc) as tc:
        with tc.tile_pool(name="const", bufs=1) as pool:
            # ---------------- constants ----------------
            pp_ctx = tc.tile_pool(name="cpsum", bufs=2, space="PSUM")
            pp = pp_ctx.__enter__()
            ident = pool.tile([P, P], F32)
            make_identity(nc, ident[:])
            ident16 = pool.tile([P, P], BF16)
            make_identity(nc, ident16[:])

            with tc.tile_pool(name="w0", bufs=1) as wp:
                at = wp.tile([BAS, R], F32)
                bs = wp.tile([BAS, IN * 16], F32)
                nc.sync.dma_start(out=at[:], in_=attT.ap()[:])
                nc.sync.dma_start(out=bs[:], in_=basis2.ap()[:])
                w5 = wp.tile([R, IN * 16], F32)
                half = IN * 16 // 2
                for h in range(2):
                    ps = pp.tile([R, half], F32, space="PSUM", tag="w5ps")
                    nc.tensor.matmul(out=ps[:], lhsT=at[:],
                                     rhs=bs[:, h * half:(h + 1) * half],
                                     start=True, stop=True)
                    nc.scalar.copy(out=w5[:, h * half:(h + 1) * half], in_=ps[:])
                nc.sync.dma_start(out=wscr.ap()[:], in_=w5[:])
            w64 = pool.tile([IN, R, 16], F32)
            nc.sync.dma_start(
                out=w64[:], in_=wscr.ap()[:].rearrange("r (k o) -> k r o", k=IN))
            wallb = pool.tile([IN, R * 16], BF16)
            nc.scalar.copy(out=wallb[:],
                           in_=w64[:].rearrange("k r o -> k (r o)"))

            fcw = pool.tile([64, MSG], F32)
            nc.sync.dma_start(out=fcw[:], in_=fc_w.ap()[:])
            psT = pp.tile([MSG, 64], F32, space="PSUM", tag="fcT")
            nc.tensor.transpose(out=psT[:], in_=fcw[:], identity=ident[:64, :64])
            fcwT = pool.tile([MSG, 64], F32)
            nc.scalar.copy(out=fcwT[:], in_=psT[:])
            fcb = pool.tile([P, 64], F32)
            nc.sync.dma_start(out=fcb[:], in_=fc_b.ap()[:].to_broadcast((P, 64)))

            cisb = pool.tile([P, 2 * TP], F32)
            nc.sync.dma_start(
                out=cisb[:],
                in_=ci_sh.ap()[:].rearrange("(t p) o -> p (t o)", p=P))
            iota = pool.tile([P, 1, P], BF16)
            nc.sync.dma_start(out=iota[:, 0, :], in_=iota_d.ap()[:])
            pp_ctx.__exit__(None, None, None)

            # ---------------- stage fsrc16 = (feat*cj) bf16 ----------------
            # Side 0 staged up front; side 1 staged during d=0's first
            # gather phase so its DMAs/DVE hide under the Pool gathers.
            GT = 8

            def stage_starts():
                starts = list(range(0, NU - GT * P + 1, GT * P))
                if NU % (GT * P):
                    starts.append(NU - GT * P)
                return starts

            def stage_group(p1, side, feat, cj, g0):
                ft = p1.tile([P, GT, IN], F32, tag="ft")
                cjt = p1.tile([P, GT, 1], F32, tag="cj")
                nc.sync.dma_start(
                    out=ft[:], in_=feat.ap()[g0:g0 + GT * P].rearrange(
                        "(p a) d -> p a d", p=P))
                nc.sync.dma_start(
                    out=cjt[:], in_=cj.ap()[g0:g0 + GT * P].rearrange(
                        "(p a) d -> p a d", p=P))
                sc = p1.tile([P, GT, IN], BF16, tag="sc")
                nc.vector.tensor_tensor(
                    out=sc[:], in0=ft[:],
                    in1=cjt[:].to_broadcast((P, GT, IN)),
                    op=ALU.mult)
                ofs = side * cfg.NUP + g0
                nc.sync.dma_start(
                    out=fsrc16.ap()[ofs:ofs + GT * P]
                        .rearrange("(p a) d -> p a d", p=P),
                    in_=sc[:])

            def stage_side(side, feat, cj):
                with tc.tile_pool(name=f"p1{side}", bufs=3) as p1:
                    for g0 in stage_starts():
                        stage_group(p1, side, feat, cj, g0)

            # Build one unit-slab of the (dd, gg) table into the DRAM stage.
            # Emitted interleaved between gather calls of the previous phase
            # so PE/DVE slices fit in per-call slack instead of front-loading
            # the engine FIFOs.
            def build_unit(dd, gg, k, tb, tbx, tbps):
                base = dd * cfg.NUP + (UN * gg + k) * SLAB
                for hh in range(2):
                    half = SLAB // 2
                    xts = tbx.tile([MSG, half], F32, tag="xts")
                    for c0 in range(0, half, 512):
                        w = min(512, half - c0)
                        fc4 = tb.tile([P, 4, IN], BF16, tag="fc4")
                        nc.sync.dma_start(
                            out=fc4[:, :w // P, :],
                            in_=fsrc16.ap()[
                                base + hh * half + c0:
                                base + hh * half + c0 + w]
                                .rearrange("(a p) d -> p a d", p=P))
                        ft4 = tb.tile([IN, 4, P], BF16, tag="ft4")
                        for j in range(w // P):
                            fps = tbps.tile([IN, P], BF16, space="PSUM",
                                            tag="fps")
                            nc.tensor.transpose(
                                out=fps[:], in_=fc4[:, j, :],
                                identity=ident16[:])
                            nc.vector.tensor_copy(
                                out=ft4[:, j, :], in_=fps[:])
                        xps = tbps.tile([MSG, 512], F32, space="PSUM",
                                        tag="xps")
                        nc.tensor.matmul(
                            out=xps[:, :w], lhsT=wallb[:],
                            rhs=ft4[:].rearrange("f a p -> f (a p)")[:, :w],
                            start=True, stop=True)
                        nc.vector.tensor_copy(
                            out=xts[:, c0:c0 + w], in_=xps[:, :w])
                    for r in range(R):
                        nc.sync.dma_start(
                            out=tblst.ap()[
                                16 * k:16 * k + 16,
                                r * SLAB + hh * (SLAB // 2):
                                r * SLAB + (hh + 1) * (SLAB // 2)],
                            in_=xts[16 * r:16 * r + 16, :])

            # ---------------- main: per direction ----------------
            g16ofs = 0
            ccofs = 0
            with tc.tile_pool(name="tblp", bufs=1) as tblp, \
                 tc.tile_pool(name="yaccp", bufs=1) as yp:
                tbl = tblp.tile([P, NELEM, 1], F32)
                from contextlib import ExitStack
                # Interleave side-0 staging with the (0,0) table build: each
                # unit-slab builds as soon as the staging groups covering its
                # slab rows have been emitted.
                with tc.tile_pool(name="p10", bufs=3) as p1s, \
                     tc.tile_pool(name="tbi", bufs=2) as tb0, \
                     tc.tile_pool(name="tbxi", bufs=1) as tbx0, \
                     tc.tile_pool(name="tbpsi", bufs=2, space="PSUM") as tbps0:
                    nextk = 0
                    for g0 in stage_starts():
                        stage_group(p1s, 0, ufeat, cj_u, g0)
                        while nextk < UN and g0 + GT * P >= (nextk + 1) * SLAB:
                            build_unit(0, 0, nextk, tb0, tbx0, tbps0)
                            nextk += 1
                    while nextk < UN:
                        build_unit(0, 0, nextk, tb0, tbx0, tbps0)
                        nextk += 1
                nc.sync.dma_start(out=tbl[:, :, 0], in_=tblst.ap()[:])
                stage_side(1, ifeat, cj_m)
                for d in range(2):
                    yacc = yp.tile([P, TP, MSG], BF16, tag="yacc")
                    nc.vector.memset(yacc[:], 0.0)
                    for g in range(2):
                        ph = 2 * d + g
                        bstack = ExitStack()
                        if ph < 3:
                            btb = bstack.enter_context(
                                tc.tile_pool(name=f"tb{ph}", bufs=2))
                            btbx = bstack.enter_context(
                                tc.tile_pool(name=f"tbx{ph}", bufs=1))
                            btbps = bstack.enter_context(
                                tc.tile_pool(name=f"tbps{ph}", bufs=1,
                                             space="PSUM"))
                        bu = 0
                        # ---- gather + segment-sum for (d, g) ----
                        chunksl = chunks_all[d * 2 + g]
                        rows = rowsdg[d * 2 + g]
                        with tc.tile_pool(name="io", bufs=2) as iop, \
                             tc.tile_pool(name="go", bufs=1) as gop, \
                             tc.tile_pool(name="gb", bufs=1) as gbp, \
                             tc.tile_pool(name="oh", bufs=2) as ohp, \
                             tc.tile_pool(name="ts", bufs=2) as tsp, \
                             tc.tile_pool(name="mps", bufs=3,
                                          space="PSUM") as mps:
                            pos = 0
                            psy = None
                            for a0 in range(0, rows, NI):
                                ni = min(NI, rows - a0)
                                gi = iop.tile([P, NI // 16], I16, tag="gi")
                                nc.sync.dma_start(
                                    out=gi[:, :ni // 16],
                                    in_=gidx.ap()[:, g16ofs + a0 // 16:
                                                  g16ofs + (a0 + ni) // 16])
                                dlt = iop.tile([P, NI // P, UN, 1], BF16,
                                               tag="dlt")
                                nc.sync.dma_start(
                                    out=dlt[:, :ni // P, :, :],
                                    in_=dl8.ap()[:, ccofs + (a0 // P) * UN:
                                                 ccofs + ((a0 + ni) // P) * UN]
                                        .rearrange("p (c k one) -> p c k one",
                                                   k=UN, one=1))
                                go = gop.tile([P, NI, 1], F32, tag="go")
                                nc.gpsimd.ap_gather(
                                    go[:, :ni, :], tbl[:], gi[:, :ni // 16],
                                    channels=P, num_elems=NELEM, d=1,
                                    num_idxs=ni)
                                gb = gbp.tile([P, NI], BF16, tag="gbc")
                                nc.vector.tensor_copy(out=gb[:, :ni],
                                                      in_=go[:, :ni, 0])
                                for local in range(ni // P):
                                    t, r, first, last = chunksl[pos]
                                    pos += 1
                                    tps = mps.tile([P, P], BF16, space="PSUM",
                                                   tag="tps")
                                    nc.tensor.transpose(
                                        out=tps[:],
                                        in_=gb[:, local * P:(local + 1) * P],
                                        identity=ident16[:])
                                    tsb = tsp.tile([P, P], BF16, tag="tsb")
                                    nc.vector.tensor_copy(out=tsb[:], in_=tps[:])
                                    oh = ohp.tile([P, UN, P], BF16, tag="oh")
                                    nc.vector.tensor_tensor(
                                        out=oh[:],
                                        in0=dlt[:, local, :, :].to_broadcast(
                                            (P, UN, P)),
                                        in1=iota[:, 0:1, :].to_broadcast(
                                            (P, UN, P)),
                                        op=ALU.is_equal)
                                    if first:
                                        psy = mps.tile([P, 16], F32,
                                                       space="PSUM", tag="psy")
                                    for k in range(UN):
                                        nc.tensor.matmul(
                                            out=psy[:],
                                            lhsT=oh[:, k, :],
                                            rhs=tsb[:, 16 * k:16 * k + 16],
                                            start=(first and k == 0),
                                            stop=(last and k == UN - 1))
                                    if last:
                                        ys = yacc[:, t, r * 16:(r + 1) * 16]
                                        nc.vector.tensor_tensor(
                                            out=ys, in0=ys, in1=psy[:],
                                            op=ALU.add)
                                if ph < 3 and a0 > 0 and bu < UN:
                                    build_unit((ph + 1) // 2, (ph + 1) % 2,
                                               bu, btb, btbx, btbps)
                                    bu += 1
                        while ph < 3 and bu < UN:
                            build_unit((ph + 1) // 2, (ph + 1) % 2,
                                       bu, btb, btbx, btbps)
                            bu += 1
                        if ph < 3:
                            # load next phase's table now, ahead of the
                            # transform DMAs in the sync-engine FIFO
                            nc.sync.dma_start(out=tbl[:, :, 0],
                                              in_=tblst.ap()[:])
                        bstack.close()
                        g16ofs += rows // 16
                        ccofs += (rows // P) * UN
                    # ---------------- transform ----------------
                    with tc.tile_pool(name="p3", bufs=3) as p3, \
                         tc.tile_pool(name="p3ps", bufs=2, space="PSUM") as p3p:
                        for t in range(TP):
                            msg = p3.tile([P, MSG], F32, tag="msg")
                            nc.scalar.activation(
                                out=msg[:], in_=yacc[:, t, :],
                                func=ACTF.Relu,
                                scale=cisb[:, d * TP + t: d * TP + t + 1])
                            psmT = p3p.tile([MSG, P], F32, space="PSUM",
                                            tag="psmT")
                            nc.tensor.transpose(out=psmT[:], in_=msg[:],
                                                identity=ident[:])
                            msgT = p3.tile([MSG, P], F32, tag="msgT")
                            nc.vector.tensor_copy(out=msgT[:], in_=psmT[:])
                            fcp = p3p.tile([P, 64], F32, space="PSUM",
                                           tag="fcp")
                            nc.tensor.matmul(
                                out=fcp[:], lhsT=msgT[:], rhs=fcwT[:],
                                start=True, stop=True)
                            osb = p3.tile([P, 64], F32, tag="osb")
                            nc.vector.tensor_tensor(out=osb[:], in0=fcp[:],
                                                    in1=fcb[:], op=ALU.add)
                            dst = m_out if d == 0 else u_out
                            nc.sync.dma_start(
                                out=dst.ap()[t * P:(t + 1) * P], in_=osb[:])
    nc.compile()
    return nc


# ----------------------------------------------------------------- kernel

def make_in_maps(cfg, gidx, dl8, inputs):
    import ml_dtypes
    ins = {k: np.asarray(v) for k, v in inputs.items()}
    iota = np.tile(np.arange(P, dtype=ml_dtypes.bfloat16), (P, 1))
    dl8 = [d.astype(ml_dtypes.bfloat16) for d in dl8]
    base = dict(
        ufeat=ins['ufeat'], ifeat=ins['ifeat'],
        cj_u=ins['cj_user'], cj_m=ins['cj_movie'],
        attT=np.ascontiguousarray(ins['att'].T),
        basis2=ins['basis'].reshape(cfg.BAS, cfg.IN * 16).copy(),
        fc_w=ins['fc_w'], fc_b=ins['fc_b'].reshape(1, 64).copy(),
        iota128=iota,
    )
    in_maps = []
    for c in range(cfg.NCORES):
        ci = np.zeros((2 * cfg.TP * P, 1), np.float32)
        ci[:cfg.NSH] = ins['ci_movie'][c * cfg.NSH:(c + 1) * cfg.NSH]
        ci[cfg.TP * P:cfg.TP * P + cfg.NSH] = \
            ins['ci_user'][c * cfg.NSH:(c + 1) * cfg.NSH]
        in_maps.append({**base, 'ci_sh': ci, 'gidx': gidx[c], 'dl8': dl8[c]})
    return in_maps


def assemble(cfg, results):
    u = np.concatenate([results[c]['u_out'][:cfg.NSH]
                        for c in range(cfg.NCORES)])
    m = np.concatenate([results[c]['m_out'][:cfg.NSH]
                        for c in range(cfg.NCORES)])
    return u, m


def kernel(**inputs):
    from concourse import bass_utils
    cfg = Cfg()
    chunks_all, rowsdg, gidx, dl8 = build_plan(cfg, inputs['edge_user'],
                                               inputs['edge_movie'])
    nc = build_program(cfg, chunks_all, rowsdg, gidx[0].shape[1],
                       dl8[0].shape[1], cfg.NCORES)
    in_maps = make_in_maps(cfg, gidx, dl8, inputs)
    res = bass_utils.run_bass_kernel_spmd(nc, in_maps,
                                          core_ids=list(range(cfg.NCORES)))
    return assemble(cfg, res.results)


# revision 29
# speedup vs baseline: 1.1885x; 1.0126x over previous
"""GCMC layer on trn2 — v2: ap_gather (GPSIMD free-dim gather) + one-hot PE
segment-sum in transformed (16-lane) message space.

Design (per device, dst-sharded: device c owns dst nodes [c*NSH, (c+1)*NSH)):
  - Transform-first: x_r = (feat*cj) @ W_r  ([N,16] per rating) so each edge
    only moves 16 lanes. Table per (side, g): SBUF [128, 32000] f32 where
    partition 16k+j = lane j of slab (8g+k) (slab=6400 srcs), elem r*6400+s.
  - Edges binned per (d, g, unit k=src slab, dst-tile t, rating r); each
    (t, r) run padded to a 128-multiple of the max count over (device, unit)
    so the SPMD program is uniform. Unit streams are position-aligned: at any
    chunk all 8 units are in the same (t, r) run.
  - nc.gpsimd.ap_gather pulls per-edge x_r lanes from the SBUF table
    (per-unit int16 idx lists; Pool engine, ~6-9ns/idx/unit, 8 units in
    parallel — replaces dma_gather's ~8.6ns/row serial descriptor gen).
  - Per 128-row position: PE transpose -> T [128 e, 128 (k,j)]; DVE builds 8
    one-hots from dloc codes; 8 matmuls accumulate psum_y[dst,16] per run;
    DVE flushes into yacc [128, TP, 80] bf16.
  - Table build: stage fsrc16=(feat*cj) bf16 in DRAM; per slab DMA-transpose
    -> FT [64, 6400]; xT = Wall^T @ FT on PE; DVE copy psum->xTslab f32;
    5 HWDGE DMAs partition-remap xTslab -> table unit block.
  - Transform per (d, tile): relu(msg*ci) on ACT, PE transpose, f32 fc matmul
    + bias, dense store.
Host only bins/sorts indices and packs int16 idx + f32 dloc blocks.
"""
import sys
import numpy as np

sys.path.insert(0, '/opt/trn_rl_repo')

import concourse.bass as bass
import concourse.tile as tile
import concourse.mybir as mybir
from concourse import bacc
from concourse.masks import make_identity

F32 = mybir.dt.float32
BF16 = mybir.dt.bfloat16
I16 = mybir.dt.int16
ALU = mybir.AluOpType
ACTF = mybir.ActivationFunctionType
P = 128


class Cfg:
    def __init__(self, NU=100000, NM=100000, R=5, E=1000000, IN=64, BAS=4,
                 NCORES=8):
        assert NU == NM
        self.NU, self.NM, self.R, self.E, self.IN, self.BAS = NU, NM, R, E, IN, BAS
        self.MPR = 16
        self.MSG = self.MPR * R              # 80
        self.OUT = 64
        self.NCORES = NCORES
        self.NSH = NU // NCORES              # dsts per device
        self.TP = -(-self.NSH // P)          # dst tiles (98)
        self.UN = 8                          # gpsimd units
        self.SLAB = 6400                     # srcs per unit-slab
        self.NG = 2                          # slab groups (16 slabs total)
        self.NELEM = self.R * self.SLAB      # 32000 table elems per partition
        self.NUP = self.SLAB * self.UN * self.NG   # padded src count 102400
        self.NI = 5120                       # ap_gather rows per call


# ----------------------------------------------------------------- host prep

def build_plan(cfg, edge_user, edge_movie):
    """Bin edges per (d, g, unit, tile, rating); pad each (t, r) run to a
    128-multiple of the max count across (device, unit).

    Returns:
      chunks: per (d, g): list of (t, r, first, last) per 128-row chunk
      rowsdg: per (d, g): padded rows per unit
      gidx:  per-device int16 [128, G16] idx blocks (concat over d, g)
      dl8:   per-device f32  [128, CC*8] dloc blocks (-1 = padding)
    """
    NC, UN, TP, R = cfg.NCORES, cfg.UN, cfg.TP, cfg.R
    NSH, SLAB = cfg.NSH, cfg.SLAB
    eu = np.asarray(edge_user)
    em = np.asarray(edge_movie)

    chunks_all = []
    rowsdg = []
    gparts = [[] for _ in range(NC)]
    dparts = [[] for _ in range(NC)]
    for d in range(2):
        src_all, dst_all = (eu, em) if d == 0 else (em, eu)
        src = src_all.reshape(-1).astype(np.int64)
        dst = dst_all.reshape(-1).astype(np.int64)
        rr = np.repeat(np.arange(R, dtype=np.int64), cfg.E)
        c = dst // NSH
        ld = dst % NSH
        t = ld // P
        dl = (ld % P).astype(np.float32)
        slab = src // SLAB
        g = slab // UN
        k = slab % UN
        s = src % SLAB
        idx = (rr * SLAB + s).astype(np.int16)
        for gv in range(2):
            m = g == gv
            key = ((c[m] * UN + k[m]) * TP + t[m]) * R + rr[m]
            cnt = np.bincount(key, minlength=NC * UN * TP * R)
            cnt = cnt.reshape(NC, UN, TP, R)
            nch = -(-cnt.max(axis=(0, 1)) // P)          # [TP, R] chunks
            L = nch * P
            Lf = L.reshape(-1)
            base = np.concatenate([[0], np.cumsum(Lf)[:-1]]).reshape(TP, R)
            rows = int(Lf.sum())
            rowsdg.append(rows)
            # rank within (c,k,t,r)
            order = np.argsort(key, kind='stable')
            ko = key[order]
            bnd = np.flatnonzero(np.diff(ko, prepend=-1))
            rank = np.arange(ko.size) - np.repeat(
                bnd, np.diff(np.append(bnd, ko.size)))
            inv = np.empty_like(order)
            inv[order] = np.arange(order.size)
            rank = rank[inv]
            pos = base[t[m], rr[m]] + rank
            gs = np.zeros((NC, UN, rows), np.int16)
            dv = np.full((NC, UN, rows), -1.0, np.float32)
            gs[c[m], k[m], pos] = idx[m]
            dv[c[m], k[m], pos] = dl[m]
            for cc in range(NC):
                gb = np.zeros((P, rows // 16), np.int16)
                db = np.zeros((P, (rows // P) * UN), np.float32)
                dbv = db.reshape(P, rows // P, UN)
                for kk in range(UN):
                    gb[16 * kk:16 * kk + 16] = \
                        gs[cc, kk].reshape(-1, 16).T
                    dbv[:, :, kk] = dv[cc, kk].reshape(-1, P).T
                gparts[cc].append(gb)
                dparts[cc].append(db)
            ch = []
            for tt in range(TP):
                for r in range(R):
                    n = int(nch[tt, r])
                    for j in range(n):
                        ch.append((tt, r, j == 0, j == n - 1))
            chunks_all.append(ch)
    gidx = [np.concatenate(gp, axis=1) for gp in gparts]
    dl8 = [np.concatenate(dp, axis=1) for dp in dparts]
    return chunks_all, rowsdg, gidx, dl8


# ------------------------------------------------------------- numpy model

def model(cfg, chunks_all, rowsdg, gidx, dl8, inputs):
    """Numpy mirror of the device program (f32, no bf16 rounding)."""
    import ml_dtypes
    BF = ml_dtypes.bfloat16
    R, NSH, TP, SLAB, UN = cfg.R, cfg.NSH, cfg.TP, cfg.SLAB, cfg.UN
    W = np.einsum('rb,bio->rio', np.asarray(inputs['att']),
                  np.asarray(inputs['basis'])).astype(np.float32)
    fc_w = np.asarray(inputs['fc_w'])
    fc_b = np.asarray(inputs['fc_b'])
    xfull = np.zeros((2, cfg.NUP, cfg.MSG), np.float32)
    for side, (f, cj) in enumerate((('ufeat', 'cj_user'), ('ifeat', 'cj_movie'))):
        fc = (np.asarray(inputs[f]) * np.asarray(inputs[cj])).astype(BF)
        for r in range(R):
            xfull[side, :cfg.NU, 16 * r:16 * r + 16] = \
                fc.astype(np.float32) @ W[r].astype(BF).astype(np.float32)
    ci = [np.asarray(inputs['ci_movie']), np.asarray(inputs['ci_user'])]

    u_out = np.zeros((cfg.NU, cfg.OUT), np.float32)
    m_out = np.zeros((cfg.NM, cfg.OUT), np.float32)
    for c in range(cfg.NCORES):
        g16o = 0
        cco = 0
        for d in range(2):
            yacc = np.zeros((TP * P, cfg.MSG), np.float32)
            for g in range(2):
                ch = chunks_all[d * 2 + g]
                rows = rowsdg[d * 2 + g]
                gb = gidx[c][:, g16o:g16o + rows // 16]
                db = dl8[c][:, cco:cco + (rows // P) * UN].reshape(
                    P, rows // P, UN)
                g16o += rows // 16
                cco += (rows // P) * UN
                trow = np.repeat([t for (t, r, _, _) in ch], P)
                rrow = np.repeat([r for (t, r, _, _) in ch], P)
                for k in range(UN):
                    idxs = gb[16 * k:16 * k + 16].T.reshape(-1)  # [rows]
                    dls = db[:, :, k].T.reshape(-1)
                    msk = dls >= 0
                    base = (8 * g + k) * SLAB
                    elem = idxs.astype(np.int64)
                    s = elem % SLAB
                    rv = elem // SLAB
                    val = np.zeros((rows, 16), np.float32)
                    sel = xfull[d, base + s]                      # [rows, 80]
                    val = sel[np.arange(rows)[:, None],
                              (rv * 16)[:, None] + np.arange(16)[None, :]]
                    tgt = trow * P + dls.astype(np.int64)
                    col = rrow * 16
                    np.add.at(yacc, (tgt[msk][:, None],
                                     (col[msk][:, None] + np.arange(16))),
                              val[msk])
            cish = np.zeros((TP * P, 1), np.float32)
            cish[:NSH] = ci[d][c * NSH:(c + 1) * NSH]
            z = np.maximum(yacc * cish, 0.0) @ fc_w.T + fc_b
            if d == 0:
                m_out[c * NSH:(c + 1) * NSH] = z[:NSH]
            else:
                u_out[c * NSH:(c + 1) * NSH] = z[:NSH]
    return u_out, m_out


# ---------------------------------------------------------- device program

def build_program(cfg, chunks_all, rowsdg, g16cols, cccols, num_devices):
    nc = bacc.Bacc("TRN2", target_bir_lowering=False, debug=False,
                   num_devices=num_devices)
    NU, IN, R, BAS = cfg.NU, cfg.IN, cfg.R, cfg.BAS
    TP, UN, SLAB, NELEM, MSG, NI = (cfg.TP, cfg.UN, cfg.SLAB, cfg.NELEM,
                                    cfg.MSG, cfg.NI)

    ufeat = nc.dram_tensor("ufeat", (NU, IN), F32, kind="ExternalInput")
    ifeat = nc.dram_tensor("ifeat", (NU, IN), F32, kind="ExternalInput")
    cj_u = nc.dram_tensor("cj_u", (NU, 1), F32, kind="ExternalInput")
    cj_m = nc.dram_tensor("cj_m", (NU, 1), F32, kind="ExternalInput")
    ci_sh = nc.dram_tensor("ci_sh", (2 * TP * P, 1), F32, kind="ExternalInput")
    attT = nc.dram_tensor("attT", (BAS, R), F32, kind="ExternalInput")
    basis2 = nc.dram_tensor("basis2", (BAS, IN * 16), F32, kind="ExternalInput")
    fc_w = nc.dram_tensor("fc_w", (64, MSG), F32, kind="ExternalInput")
    fc_b = nc.dram_tensor("fc_b", (1, 64), F32, kind="ExternalInput")
    iota_d = nc.dram_tensor("iota128", (P, P), BF16, kind="ExternalInput")
    gidx = nc.dram_tensor("gidx", (P, g16cols), I16, kind="ExternalInput")
    dl8 = nc.dram_tensor("dl8", (P, cccols), BF16, kind="ExternalInput")

    m_out = nc.dram_tensor("m_out", (TP * P, 64), F32, kind="ExternalOutput")
    u_out = nc.dram_tensor("u_out", (TP * P, 64), F32, kind="ExternalOutput")

    wscr = nc.dram_tensor("wscr", (R, IN * 16), F32, kind="Internal")
    fsrc16 = nc.dram_tensor("fsrc16", (2 * cfg.NUP, IN), BF16, kind="Internal")
    tblst = nc.dram_tensor("tblst", (P, NELEM), F32, kind="Internal")

    with tile.TileContext(nc) as tc:
        with tc.tile_pool(name="const", bufs=1) as pool:
            # ---------------- constants ----------------
            pp_ctx = tc.tile_pool(name="cpsum", bufs=2, space="PSUM")
            pp = pp_ctx.__enter__()
            ident = pool.tile([P, P], F32)
            make_identity(nc, ident[:])
            ident16 = pool.tile([P, P], BF16)
            make_identity(nc, ident16[:])

            with tc.tile_pool(name="w0", bufs=1) as wp:
                at = wp.tile([BAS, R], F32)
                bs = wp.tile([BAS, IN * 16], F32)
                nc.sync.dma_start(out=at[:], in_=attT.ap()[:])
                nc.sync.dma_start(out=bs[:], in_=basis2.ap()[:])
                w5 = wp.tile([R, IN * 16], F32)
                half = IN * 16 // 2
                for h in range(2):
                    ps = pp.tile([R, half], F32, space="PSUM", tag="w5ps")
                    nc.tensor.matmul(out=ps[:], lhsT=at[:],
                                     rhs=bs[:, h * half:(h + 1) * half],
                                     start=True, stop=True)
                    nc.scalar.copy(out=w5[:, h * half:(h + 1) * half], in_=ps[:])
                nc.sync.dma_start(out=wscr.ap()[:], in_=w5[:])
            w64 = pool.tile([IN, R, 16], F32)
            nc.sync.dma_start(
                out=w64[:], in_=wscr.ap()[:].rearrange("r (k o) -> k r o", k=IN))
            wallb = pool.tile([IN, R * 16], BF16)
            nc.scalar.copy(out=wallb[:],
                           in_=w64[:].rearrange("k r o -> k (r o)"))

            fcw = pool.tile([64, MSG], F32)
            nc.sync.dma_start(out=fcw[:], in_=fc_w.ap()[:])
            psT = pp.tile([MSG, 64], F32, space="PSUM", tag="fcT")
            nc.tensor.transpose(out=psT[:], in_=fcw[:], identity=ident[:64, :64])
            fcwT = pool.tile([MSG, 64], F32)
            nc.scalar.copy(out=fcwT[:], in_=psT[:])
            fcb = pool.tile([P, 64], F32)
            nc.sync.dma_start(out=fcb[:], in_=fc_b.ap()[:].to_broadcast((P, 64)))

            cisb = pool.tile([P, 2 * TP], F32)
            nc.sync.dma_start(
                out=cisb[:],
                in_=ci_sh.ap()[:].rearrange("(t p) o -> p (t o)", p=P))
            iota = pool.tile([P, 1, P], BF16)
            nc.sync.dma_start(out=iota[:, 0, :], in_=iota_d.ap()[:])
            pp_ctx.__exit__(None, None, None)

            # ---------------- stage fsrc16 = (feat*cj) bf16 ----------------
            # Side 0 staged up front; side 1 staged during d=0's first
            # gather phase so its DMAs/DVE hide under the Pool gathers.
            GT = 8

            def stage_starts():
                starts = list(range(0, NU - GT * P + 1, GT * P))
                if NU % (GT * P):
                    starts.append(NU - GT * P)
                return starts

            def stage_group(p1, side, feat, cj, g0):
                ft = p1.tile([P, GT, IN], F32, tag="ft")
                cjt = p1.tile([P, GT, 1], F32, tag="cj")
                nc.scalar.dma_start(
                    out=ft[:], in_=feat.ap()[g0:g0 + GT * P].rearrange(
                        "(p a) d -> p a d", p=P))
                nc.scalar.dma_start(
                    out=cjt[:], in_=cj.ap()[g0:g0 + GT * P].rearrange(
                        "(p a) d -> p a d", p=P))
                sc = p1.tile([P, GT, IN], BF16, tag="sc")
                nc.vector.tensor_tensor(
                    out=sc[:], in0=ft[:],
                    in1=cjt[:].to_broadcast((P, GT, IN)),
                    op=ALU.mult)
                ofs = side * cfg.NUP + g0
                nc.sync.dma_start(
                    out=fsrc16.ap()[ofs:ofs + GT * P]
                        .rearrange("(p a) d -> p a d", p=P),
                    in_=sc[:])

            def stage_side(side, feat, cj):
                with tc.tile_pool(name=f"p1{side}", bufs=3) as p1:
                    for g0 in stage_starts():
                        stage_group(p1, side, feat, cj, g0)

            # Build one unit-slab of the (dd, gg) table into the DRAM stage.
            # Emitted interleaved between gather calls of the previous phase
            # so PE/DVE slices fit in per-call slack instead of front-loading
            # the engine FIFOs.
            def build_unit(dd, gg, k, tb, tbx, tbps):
                base = dd * cfg.NUP + (UN * gg + k) * SLAB
                for hh in range(2):
                    half = SLAB // 2
                    xts = tbx.tile([MSG, half], F32, tag="xts")
                    for c0 in range(0, half, 512):
                        w = min(512, half - c0)
                        fc4 = tb.tile([P, 4, IN], BF16, tag="fc4")
                        nc.scalar.dma_start(
                            out=fc4[:, :w // P, :],
                            in_=fsrc16.ap()[
                                base + hh * half + c0:
                                base + hh * half + c0 + w]
                                .rearrange("(a p) d -> p a d", p=P))
                        ft4 = tb.tile([IN, 4, P], BF16, tag="ft4")
                        for j in range(w // P):
                            fps = tbps.tile([IN, P], BF16, space="PSUM",
                                            tag="fps")
                            nc.tensor.transpose(
                                out=fps[:], in_=fc4[:, j, :],
                                identity=ident16[:])
                            nc.vector.tensor_copy(
                                out=ft4[:, j, :], in_=fps[:])
                        xps = tbps.tile([MSG, 512], F32, space="PSUM",
                                        tag="xps")
                        nc.tensor.matmul(
                            out=xps[:, :w], lhsT=wallb[:],
                            rhs=ft4[:].rearrange("f a p -> f (a p)")[:, :w],
                            start=True, stop=True)
                        nc.vector.tensor_copy(
                            out=xts[:, c0:c0 + w], in_=xps[:, :w])
                    for r in range(R):
                        nc.sync.dma_start(
                            out=tblst.ap()[
                                16 * k:16 * k + 16,
                                r * SLAB + hh * (SLAB // 2):
                                r * SLAB + (hh + 1) * (SLAB // 2)],
                            in_=xts[16 * r:16 * r + 16, :])

            # ---------------- main: per direction ----------------
            g16ofs = 0
            ccofs = 0
            with tc.tile_pool(name="tblp", bufs=1) as tblp, \
                 tc.tile_pool(name="yaccp", bufs=1) as yp:
                tbl = tblp.tile([P, NELEM, 1], F32)
                from contextlib import ExitStack
                # Interleave side-0 staging with the (0,0) table build: each
                # unit-slab builds as soon as the staging groups covering its
                # slab rows have been emitted.
                with tc.tile_pool(name="p10", bufs=3) as p1s, \
                     tc.tile_pool(name="tbi", bufs=2) as tb0, \
                     tc.tile_pool(name="tbxi", bufs=1) as tbx0, \
                     tc.tile_pool(name="tbpsi", bufs=2, space="PSUM") as tbps0:
                    nextk = 0
                    for g0 in stage_starts():
                        stage_group(p1s, 0, ufeat, cj_u, g0)
                        while nextk < UN and g0 + GT * P >= (nextk + 1) * SLAB:
                            build_unit(0, 0, nextk, tb0, tbx0, tbps0)
                            nextk += 1
                    while nextk < UN:
                        build_unit(0, 0, nextk, tb0, tbx0, tbps0)
                        nextk += 1
                nc.sync.dma_start(out=tbl[:, :, 0], in_=tblst.ap()[:])
                stage_side(1, ifeat, cj_m)
                for d in range(2):
                    yacc = yp.tile([P, TP, MSG], BF16, tag="yacc")
                    nc.vector.memset(yacc[:], 0.0)
                    for g in range(2):
                        ph = 2 * d + g
                        bstack = ExitStack()
                        if ph < 3:
                            btb = bstack.enter_context(
                                tc.tile_pool(name=f"tb{ph}", bufs=2))
                            btbx = bstack.enter_context(
                                tc.tile_pool(name=f"tbx{ph}", bufs=1))
                            btbps = bstack.enter_context(
                                tc.tile_pool(name=f"tbps{ph}", bufs=1,
                                             space="PSUM"))
                        bu = 0
                        # ---- gather + segment-sum for (d, g) ----
                        chunksl = chunks_all[d * 2 + g]
                        rows = rowsdg[d * 2 + g]
                        with tc.tile_pool(name="io", bufs=2) as iop, \
                             tc.tile_pool(name="go", bufs=1) as gop, \
                             tc.tile_pool(name="gb", bufs=1) as gbp, \
                             tc.tile_pool(name="oh", bufs=2) as ohp, \
                             tc.tile_pool(name="ts", bufs=2) as tsp, \
                             tc.tile_pool(name="mps", bufs=3,
                                          space="PSUM") as mps:
                            pos = 0
                            psy = None
                            for a0 in range(0, rows, NI):
                                ni = min(NI, rows - a0)
                                gi = iop.tile([P, NI // 16], I16, tag="gi")
                                nc.scalar.dma_start(
                                    out=gi[:, :ni // 16],
                                    in_=gidx.ap()[:, g16ofs + a0 // 16:
                                                  g16ofs + (a0 + ni) // 16])
                                dlt = iop.tile([P, NI // P, UN, 1], BF16,
                                               tag="dlt")
                                nc.scalar.dma_start(
                                    out=dlt[:, :ni // P, :, :],
                                    in_=dl8.ap()[:, ccofs + (a0 // P) * UN:
                                                 ccofs + ((a0 + ni) // P) * UN]
                                        .rearrange("p (c k one) -> p c k one",
                                                   k=UN, one=1))
                                go = gop.tile([P, NI, 1], F32, tag="go")
                                nc.gpsimd.ap_gather(
                                    go[:, :ni, :], tbl[:], gi[:, :ni // 16],
                                    channels=P, num_elems=NELEM, d=1,
                                    num_idxs=ni)
                                gb = gbp.tile([P, NI], BF16, tag="gbc")
                                nc.vector.tensor_copy(out=gb[:, :ni],
                                                      in_=go[:, :ni, 0])
                                for local in range(ni // P):
                                    t, r, first, last = chunksl[pos]
                                    pos += 1
                                    tps = mps.tile([P, P], BF16, space="PSUM",
                                                   tag="tps")
                                    nc.tensor.transpose(
                                        out=tps[:],
                                        in_=gb[:, local * P:(local + 1) * P],
                                        identity=ident16[:])
                                    tsb = tsp.tile([P, P], BF16, tag="tsb")
                                    nc.vector.tensor_copy(out=tsb[:], in_=tps[:])
                                    oh = ohp.tile([P, UN, P], BF16, tag="oh")
                                    nc.vector.tensor_tensor(
                                        out=oh[:],
                                        in0=dlt[:, local, :, :].to_broadcast(
                                            (P, UN, P)),
                                        in1=iota[:, 0:1, :].to_broadcast(
                                            (P, UN, P)),
                                        op=ALU.is_equal)
                                    if first:
                                        psy = mps.tile([P, 16], F32,
                                                       space="PSUM", tag="psy")
                                    for k in range(UN):
                                        nc.tensor.matmul(
                                            out=psy[:],
                                            lhsT=oh[:, k, :],
                                            rhs=tsb[:, 16 * k:16 * k + 16],
                                            start=(first and k == 0),
                                            stop=(last and k == UN - 1))
                                    if last:
                                        ys = yacc[:, t, r * 16:(r + 1) * 16]
                                        nc.vector.tensor_tensor(
                                            out=ys, in0=ys, in1=psy[:],
                                            op=ALU.add)
                                if ph < 3 and a0 > 0 and bu < UN:
                                    build_unit((ph + 1) // 2, (ph + 1) % 2,
                                               bu, btb, btbx, btbps)
                                    bu += 1
                        while ph < 3 and bu < UN:
                            build_unit((ph + 1) // 2, (ph + 1) % 2,
                                       bu, btb, btbx, btbps)
                            bu += 1
                        if ph < 3:
                            # load next phase's table now, ahead of the
                            # transform DMAs in the sync-engine FIFO
                            nc.sync.dma_start(out=tbl[:, :, 0],
                                              in_=tblst.ap()[:])
                        bstack.close()
                        g16ofs += rows // 16
                        ccofs += (rows // P) * UN
                    # ---------------- transform ----------------
                    with tc.tile_pool(name="p3", bufs=3) as p3, \
                         tc.tile_pool(name="p3ps", bufs=2, space="PSUM") as p3p:
                        for t in range(TP):
                            msg = p3.tile([P, MSG], F32, tag="msg")
                            nc.scalar.activation(
                                out=msg[:], in_=yacc[:, t, :],
                                func=ACTF.Relu,
                                scale=cisb[:, d * TP + t: d * TP + t + 1])
                            psmT = p3p.tile([MSG, P], F32, space="PSUM",
                                            tag="psmT")
                            nc.tensor.transpose(out=psmT[:], in_=msg[:],
                                                identity=ident[:])
                            msgT = p3.tile([MSG, P], F32, tag="msgT")
                            nc.vector.tensor_copy(out=msgT[:], in_=psmT[:])
                            fcp = p3p.tile([P, 64], F32, space="PSUM",
                                           tag="fcp")
                            nc.tensor.matmul(
                                out=fcp[:], lhsT=msgT[:], rhs=fcwT[:],
                                start=True, stop=True)
                            osb = p3.tile([P, 64], F32, tag="osb")
                            nc.vector.tensor_tensor(out=osb[:], in0=fcp[:],
                                                    in1=fcb[:], op=ALU.add)
                            dst = m_out if d == 0 else u_out
                            nc.sync.dma_start(
                                out=dst.ap()[t * P:(t + 1) * P], in_=osb[:])
    nc.compile()
    return nc


# ----------------------------------------------------------------- kernel

def make_in_maps(cfg, gidx, dl8, inputs):
    import ml_dtypes
    ins = {k: np.asarray(v) for k, v in inputs.items()}
    iota = np.tile(np.arange(P, dtype=ml_dtypes.bfloat16), (P, 1))
    dl8 = [d.astype(ml_dtypes.bfloat16) for d in dl8]
    base = dict(
        ufeat=ins['ufeat'], ifeat=ins['ifeat'],
        cj_u=ins['cj_user'], cj_m=ins['cj_movie'],
        attT=np.ascontiguousarray(ins['att'].T),
        basis2=ins['basis'].reshape(cfg.BAS, cfg.IN * 16).copy(),
        fc_w=ins['fc_w'], fc_b=ins['fc_b'].reshape(1, 64).copy(),
        iota128=iota,
    )
    in_maps = []
    for c in range(cfg.NCORES):
        ci = np.zeros((2 * cfg.TP * P, 1), np.float32)
        ci[:cfg.NSH] = ins['ci_movie'][c * cfg.NSH:(c + 1) * cfg.NSH]
        ci[cfg.TP * P:cfg.TP * P + cfg.NSH] = \
            ins['ci_user'][c * cfg.NSH:(c + 1) * cfg.NSH]
        in_maps.append({**base, 'ci_sh': ci, 'gidx': gidx[c], 'dl8': dl8[c]})
    return in_maps


def assemble(cfg, results):
    u = np.concatenate([results[c]['u_out'][:cfg.NSH]
                        for c in range(cfg.NCORES)])
    m = np.concatenate([results[c]['m_out'][:cfg.NSH]
                        for c in range(cfg.NCORES)])
    return u, m


def kernel(**inputs):
    from concourse import bass_utils
    cfg = Cfg()
    chunks_all, rowsdg, gidx, dl8 = build_plan(cfg, inputs['edge_user'],
                                               inputs['edge_movie'])
    nc = build_program(cfg, chunks_all, rowsdg, gidx[0].shape[1],
                       dl8[0].shape[1], cfg.NCORES)
    in_maps = make_in_maps(cfg, gidx, dl8, inputs)
    res = bass_utils.run_bass_kernel_spmd(nc, in_maps,
                                          core_ids=list(range(cfg.NCORES)))
    return assemble(cfg, res.results)


# revision 30
# speedup vs baseline: 1.2166x; 1.0236x over previous
"""GCMC layer on trn2 — v2: ap_gather (GPSIMD free-dim gather) + one-hot PE
segment-sum in transformed (16-lane) message space.

Design (per device, dst-sharded: device c owns dst nodes [c*NSH, (c+1)*NSH)):
  - Transform-first: x_r = (feat*cj) @ W_r  ([N,16] per rating) so each edge
    only moves 16 lanes. Table per (side, g): SBUF [128, 32000] f32 where
    partition 16k+j = lane j of slab (8g+k) (slab=6400 srcs), elem r*6400+s.
  - Edges binned per (d, g, unit k=src slab, dst-tile t, rating r); each
    (t, r) run padded to a 128-multiple of the max count over (device, unit)
    so the SPMD program is uniform. Unit streams are position-aligned: at any
    chunk all 8 units are in the same (t, r) run.
  - nc.gpsimd.ap_gather pulls per-edge x_r lanes from the SBUF table
    (per-unit int16 idx lists; Pool engine, ~6-9ns/idx/unit, 8 units in
    parallel — replaces dma_gather's ~8.6ns/row serial descriptor gen).
  - Per 128-row position: PE transpose -> T [128 e, 128 (k,j)]; DVE builds 8
    one-hots from dloc codes; 8 matmuls accumulate psum_y[dst,16] per run;
    DVE flushes into yacc [128, TP, 80] bf16.
  - Table build: stage fsrc16=(feat*cj) bf16 in DRAM; per slab DMA-transpose
    -> FT [64, 6400]; xT = Wall^T @ FT on PE; DVE copy psum->xTslab f32;
    5 HWDGE DMAs partition-remap xTslab -> table unit block.
  - Transform per (d, tile): relu(msg*ci) on ACT, PE transpose, f32 fc matmul
    + bias, dense store.
Host only bins/sorts indices and packs int16 idx + f32 dloc blocks.
"""
import sys
import numpy as np

sys.path.insert(0, '/opt/trn_rl_repo')

import concourse.bass as bass
import concourse.tile as tile
import concourse.mybir as mybir
from concourse import bacc
from concourse.masks import make_identity

F32 = mybir.dt.float32
BF16 = mybir.dt.bfloat16
I16 = mybir.dt.int16
ALU = mybir.AluOpType
ACTF = mybir.ActivationFunctionType
P = 128


class Cfg:
    def __init__(self, NU=100000, NM=100000, R=5, E=1000000, IN=64, BAS=4,
                 NCORES=8):
        assert NU == NM
        self.NU, self.NM, self.R, self.E, self.IN, self.BAS = NU, NM, R, E, IN, BAS
        self.MPR = 16
        self.MSG = self.MPR * R              # 80
        self.OUT = 64
        self.NCORES = NCORES
        self.NSH = NU // NCORES              # dsts per device
        self.TP = -(-self.NSH // P)          # dst tiles (98)
        self.UN = 8                          # gpsimd units
        self.SLAB = 6400                     # srcs per unit-slab
        self.NG = 2                          # slab groups (16 slabs total)
        self.NELEM = self.R * self.SLAB      # 32000 table elems per partition
        self.NUP = self.SLAB * self.UN * self.NG   # padded src count 102400
        self.NI = 3072                       # ap_gather rows per call


# ----------------------------------------------------------------- host prep

def build_plan(cfg, edge_user, edge_movie):
    """Bin edges per (d, g, unit, tile, rating); pad each (t, r) run to a
    128-multiple of the max count across (device, unit).

    Returns:
      chunks: per (d, g): list of (t, r, first, last) per 128-row chunk
      rowsdg: per (d, g): padded rows per unit
      gidx:  per-device int16 [128, G16] idx blocks (concat over d, g)
      dl8:   per-device f32  [128, CC*8] dloc blocks (-1 = padding)
    """
    NC, UN, TP, R = cfg.NCORES, cfg.UN, cfg.TP, cfg.R
    NSH, SLAB = cfg.NSH, cfg.SLAB
    eu = np.asarray(edge_user)
    em = np.asarray(edge_movie)

    chunks_all = []
    rowsdg = []
    gparts = [[] for _ in range(NC)]
    dparts = [[] for _ in range(NC)]
    for d in range(2):
        src_all, dst_all = (eu, em) if d == 0 else (em, eu)
        src = src_all.reshape(-1).astype(np.int64)
        dst = dst_all.reshape(-1).astype(np.int64)
        rr = np.repeat(np.arange(R, dtype=np.int64), cfg.E)
        c = dst // NSH
        ld = dst % NSH
        t = ld // P
        dl = (ld % P).astype(np.float32)
        slab = src // SLAB
        g = slab // UN
        k = slab % UN
        s = src % SLAB
        idx = (rr * SLAB + s).astype(np.int16)
        for gv in range(2):
            m = g == gv
            key = ((c[m] * UN + k[m]) * TP + t[m]) * R + rr[m]
            cnt = np.bincount(key, minlength=NC * UN * TP * R)
            cnt = cnt.reshape(NC, UN, TP, R)
            nch = -(-cnt.max(axis=(0, 1)) // P)          # [TP, R] chunks
            L = nch * P
            Lf = L.reshape(-1)
            base = np.concatenate([[0], np.cumsum(Lf)[:-1]]).reshape(TP, R)
            rows = int(Lf.sum())
            rowsdg.append(rows)
            # rank within (c,k,t,r)
            order = np.argsort(key, kind='stable')
            ko = key[order]
            bnd = np.flatnonzero(np.diff(ko, prepend=-1))
            rank = np.arange(ko.size) - np.repeat(
                bnd, np.diff(np.append(bnd, ko.size)))
            inv = np.empty_like(order)
            inv[order] = np.arange(order.size)
            rank = rank[inv]
            pos = base[t[m], rr[m]] + rank
            gs = np.zeros((NC, UN, rows), np.int16)
            dv = np.full((NC, UN, rows), -1.0, np.float32)
            gs[c[m], k[m], pos] = idx[m]
            dv[c[m], k[m], pos] = dl[m]
            for cc in range(NC):
                gb = np.zeros((P, rows // 16), np.int16)
                db = np.zeros((P, (rows // P) * UN), np.float32)
                dbv = db.reshape(P, rows // P, UN)
                for kk in range(UN):
                    gb[16 * kk:16 * kk + 16] = \
                        gs[cc, kk].reshape(-1, 16).T
                    dbv[:, :, kk] = dv[cc, kk].reshape(-1, P).T
                gparts[cc].append(gb)
                dparts[cc].append(db)
            ch = []
            for tt in range(TP):
                for r in range(R):
                    n = int(nch[tt, r])
                    for j in range(n):
                        ch.append((tt, r, j == 0, j == n - 1))
            chunks_all.append(ch)
    gidx = [np.concatenate(gp, axis=1) for gp in gparts]
    dl8 = [np.concatenate(dp, axis=1) for dp in dparts]
    return chunks_all, rowsdg, gidx, dl8


# ------------------------------------------------------------- numpy model

def model(cfg, chunks_all, rowsdg, gidx, dl8, inputs):
    """Numpy mirror of the device program (f32, no bf16 rounding)."""
    import ml_dtypes
    BF = ml_dtypes.bfloat16
    R, NSH, TP, SLAB, UN = cfg.R, cfg.NSH, cfg.TP, cfg.SLAB, cfg.UN
    W = np.einsum('rb,bio->rio', np.asarray(inputs['att']),
                  np.asarray(inputs['basis'])).astype(np.float32)
    fc_w = np.asarray(inputs['fc_w'])
    fc_b = np.asarray(inputs['fc_b'])
    xfull = np.zeros((2, cfg.NUP, cfg.MSG), np.float32)
    for side, (f, cj) in enumerate((('ufeat', 'cj_user'), ('ifeat', 'cj_movie'))):
        fc = (np.asarray(inputs[f]) * np.asarray(inputs[cj])).astype(BF)
        for r in range(R):
            xfull[side, :cfg.NU, 16 * r:16 * r + 16] = \
                fc.astype(np.float32) @ W[r].astype(BF).astype(np.float32)
    ci = [np.asarray(inputs['ci_movie']), np.asarray(inputs['ci_user'])]

    u_out = np.zeros((cfg.NU, cfg.OUT), np.float32)
    m_out = np.zeros((cfg.NM, cfg.OUT), np.float32)
    for c in range(cfg.NCORES):
        g16o = 0
        cco = 0
        for d in range(2):
            yacc = np.zeros((TP * P, cfg.MSG), np.float32)
            for g in range(2):
                ch = chunks_all[d * 2 + g]
                rows = rowsdg[d * 2 + g]
                gb = gidx[c][:, g16o:g16o + rows // 16]
                db = dl8[c][:, cco:cco + (rows // P) * UN].reshape(
                    P, rows // P, UN)
                g16o += rows // 16
                cco += (rows // P) * UN
                trow = np.repeat([t for (t, r, _, _) in ch], P)
                rrow = np.repeat([r for (t, r, _, _) in ch], P)
                for k in range(UN):
                    idxs = gb[16 * k:16 * k + 16].T.reshape(-1)  # [rows]
                    dls = db[:, :, k].T.reshape(-1)
                    msk = dls >= 0
                    base = (8 * g + k) * SLAB
                    elem = idxs.astype(np.int64)
                    s = elem % SLAB
                    rv = elem // SLAB
                    val = np.zeros((rows, 16), np.float32)
                    sel = xfull[d, base + s]                      # [rows, 80]
                    val = sel[np.arange(rows)[:, None],
                              (rv * 16)[:, None] + np.arange(16)[None, :]]
                    tgt = trow * P + dls.astype(np.int64)
                    col = rrow * 16
                    np.add.at(yacc, (tgt[msk][:, None],
                                     (col[msk][:, None] + np.arange(16))),
                              val[msk])
            cish = np.zeros((TP * P, 1), np.float32)
            cish[:NSH] = ci[d][c * NSH:(c + 1) * NSH]
            z = np.maximum(yacc * cish, 0.0) @ fc_w.T + fc_b
            if d == 0:
                m_out[c * NSH:(c + 1) * NSH] = z[:NSH]
            else:
                u_out[c * NSH:(c + 1) * NSH] = z[:NSH]
    return u_out, m_out


# ---------------------------------------------------------- device program

def build_program(cfg, chunks_all, rowsdg, g16cols, cccols, num_devices):
    nc = bacc.Bacc("TRN2", target_bir_lowering=False, debug=False,
                   num_devices=num_devices)
    NU, IN, R, BAS = cfg.NU, cfg.IN, cfg.R, cfg.BAS
    TP, UN, SLAB, NELEM, MSG, NI = (cfg.TP, cfg.UN, cfg.SLAB, cfg.NELEM,
                                    cfg.MSG, cfg.NI)

    ufeat = nc.dram_tensor("ufeat", (NU, IN), F32, kind="ExternalInput")
    ifeat = nc.dram_tensor("ifeat", (NU, IN), F32, kind="ExternalInput")
    cj_u = nc.dram_tensor("cj_u", (NU, 1), F32, kind="ExternalInput")
    cj_m = nc.dram_tensor("cj_m", (NU, 1), F32, kind="ExternalInput")
    ci_sh = nc.dram_tensor("ci_sh", (2 * TP * P, 1), F32, kind="ExternalInput")
    attT = nc.dram_tensor("attT", (BAS, R), F32, kind="ExternalInput")
    basis2 = nc.dram_tensor("basis2", (BAS, IN * 16), F32, kind="ExternalInput")
    fc_w = nc.dram_tensor("fc_w", (64, MSG), F32, kind="ExternalInput")
    fc_b = nc.dram_tensor("fc_b", (1, 64), F32, kind="ExternalInput")
    iota_d = nc.dram_tensor("iota128", (P, P), BF16, kind="ExternalInput")
    gidx = nc.dram_tensor("gidx", (P, g16cols), I16, kind="ExternalInput")
    dl8 = nc.dram_tensor("dl8", (P, cccols), BF16, kind="ExternalInput")

    m_out = nc.dram_tensor("m_out", (TP * P, 64), F32, kind="ExternalOutput")
    u_out = nc.dram_tensor("u_out", (TP * P, 64), F32, kind="ExternalOutput")

    wscr = nc.dram_tensor("wscr", (R, IN * 16), F32, kind="Internal")
    fsrc16 = nc.dram_tensor("fsrc16", (2 * cfg.NUP, IN), BF16, kind="Internal")
    tblst = nc.dram_tensor("tblst", (P, NELEM), F32, kind="Internal")

    with tile.TileContext(nc) as tc:
        with tc.tile_pool(name="const", bufs=1) as pool:
            # ---------------- constants ----------------
            pp_ctx = tc.tile_pool(name="cpsum", bufs=2, space="PSUM")
            pp = pp_ctx.__enter__()
            ident = pool.tile([P, P], F32)
            make_identity(nc, ident[:])
            ident16 = pool.tile([P, P], BF16)
            make_identity(nc, ident16[:])

            with tc.tile_pool(name="w0", bufs=1) as wp:
                at = wp.tile([BAS, R], F32)
                bs = wp.tile([BAS, IN * 16], F32)
                nc.sync.dma_start(out=at[:], in_=attT.ap()[:])
                nc.sync.dma_start(out=bs[:], in_=basis2.ap()[:])
                w5 = wp.tile([R, IN * 16], F32)
                half = IN * 16 // 2
                for h in range(2):
                    ps = pp.tile([R, half], F32, space="PSUM", tag="w5ps")
                    nc.tensor.matmul(out=ps[:], lhsT=at[:],
                                     rhs=bs[:, h * half:(h + 1) * half],
                                     start=True, stop=True)
                    nc.scalar.copy(out=w5[:, h * half:(h + 1) * half], in_=ps[:])
                nc.sync.dma_start(out=wscr.ap()[:], in_=w5[:])
            w64 = pool.tile([IN, R, 16], F32)
            nc.sync.dma_start(
                out=w64[:], in_=wscr.ap()[:].rearrange("r (k o) -> k r o", k=IN))
            wallb = pool.tile([IN, R * 16], BF16)
            nc.scalar.copy(out=wallb[:],
                           in_=w64[:].rearrange("k r o -> k (r o)"))

            fcw = pool.tile([64, MSG], F32)
            nc.sync.dma_start(out=fcw[:], in_=fc_w.ap()[:])
            psT = pp.tile([MSG, 64], F32, space="PSUM", tag="fcT")
            nc.tensor.transpose(out=psT[:], in_=fcw[:], identity=ident[:64, :64])
            fcwT = pool.tile([MSG, 64], F32)
            nc.scalar.copy(out=fcwT[:], in_=psT[:])
            fcb = pool.tile([P, 64], F32)
            nc.sync.dma_start(out=fcb[:], in_=fc_b.ap()[:].to_broadcast((P, 64)))

            cisb = pool.tile([P, 2 * TP], F32)
            nc.sync.dma_start(
                out=cisb[:],
                in_=ci_sh.ap()[:].rearrange("(t p) o -> p (t o)", p=P))
            iota = pool.tile([P, 1, P], BF16)
            nc.sync.dma_start(out=iota[:, 0, :], in_=iota_d.ap()[:])
            pp_ctx.__exit__(None, None, None)

            # ---------------- stage fsrc16 = (feat*cj) bf16 ----------------
            # Side 0 staged up front; side 1 staged during d=0's first
            # gather phase so its DMAs/DVE hide under the Pool gathers.
            GT = 8

            def stage_starts():
                starts = list(range(0, NU - GT * P + 1, GT * P))
                if NU % (GT * P):
                    starts.append(NU - GT * P)
                return starts

            def stage_group(p1, side, feat, cj, g0):
                ft = p1.tile([P, GT, IN], F32, tag="ft")
                cjt = p1.tile([P, GT, 1], F32, tag="cj")
                nc.scalar.dma_start(
                    out=ft[:], in_=feat.ap()[g0:g0 + GT * P].rearrange(
                        "(p a) d -> p a d", p=P))
                nc.scalar.dma_start(
                    out=cjt[:], in_=cj.ap()[g0:g0 + GT * P].rearrange(
                        "(p a) d -> p a d", p=P))
                sc = p1.tile([P, GT, IN], BF16, tag="sc")
                nc.vector.tensor_tensor(
                    out=sc[:], in0=ft[:],
                    in1=cjt[:].to_broadcast((P, GT, IN)),
                    op=ALU.mult)
                ofs = side * cfg.NUP + g0
                nc.sync.dma_start(
                    out=fsrc16.ap()[ofs:ofs + GT * P]
                        .rearrange("(p a) d -> p a d", p=P),
                    in_=sc[:])

            def stage_side(side, feat, cj):
                with tc.tile_pool(name=f"p1{side}", bufs=3) as p1:
                    for g0 in stage_starts():
                        stage_group(p1, side, feat, cj, g0)

            # Build one unit-slab of the (dd, gg) table into the DRAM stage.
            # Emitted interleaved between gather calls of the previous phase
            # so PE/DVE slices fit in per-call slack instead of front-loading
            # the engine FIFOs.
            def build_unit(dd, gg, k, tb, tbx, tbps):
                base = dd * cfg.NUP + (UN * gg + k) * SLAB
                for hh in range(2):
                    half = SLAB // 2
                    xts = tbx.tile([MSG, half], F32, tag="xts")
                    for c0 in range(0, half, 512):
                        w = min(512, half - c0)
                        fc4 = tb.tile([P, 4, IN], BF16, tag="fc4")
                        nc.scalar.dma_start(
                            out=fc4[:, :w // P, :],
                            in_=fsrc16.ap()[
                                base + hh * half + c0:
                                base + hh * half + c0 + w]
                                .rearrange("(a p) d -> p a d", p=P))
                        ft4 = tb.tile([IN, 4, P], BF16, tag="ft4")
                        for j in range(w // P):
                            fps = tbps.tile([IN, P], BF16, space="PSUM",
                                            tag="fps")
                            nc.tensor.transpose(
                                out=fps[:], in_=fc4[:, j, :],
                                identity=ident16[:])
                            nc.vector.tensor_copy(
                                out=ft4[:, j, :], in_=fps[:])
                        xps = tbps.tile([MSG, 512], F32, space="PSUM",
                                        tag="xps")
                        nc.tensor.matmul(
                            out=xps[:, :w], lhsT=wallb[:],
                            rhs=ft4[:].rearrange("f a p -> f (a p)")[:, :w],
                            start=True, stop=True)
                        nc.vector.tensor_copy(
                            out=xts[:, c0:c0 + w], in_=xps[:, :w])
                    for r in range(R):
                        nc.sync.dma_start(
                            out=tblst.ap()[
                                16 * k:16 * k + 16,
                                r * SLAB + hh * (SLAB // 2):
                                r * SLAB + (hh + 1) * (SLAB // 2)],
                            in_=xts[16 * r:16 * r + 16, :])

            # ---------------- main: per direction ----------------
            g16ofs = 0
            ccofs = 0
            with tc.tile_pool(name="tblp", bufs=1) as tblp, \
                 tc.tile_pool(name="yaccp", bufs=1) as yp:
                tbl = tblp.tile([P, NELEM, 1], F32)
                from contextlib import ExitStack
                # Interleave side-0 staging with the (0,0) table build: each
                # unit-slab builds as soon as the staging groups covering its
                # slab rows have been emitted.
                with tc.tile_pool(name="p10", bufs=3) as p1s, \
                     tc.tile_pool(name="tbi", bufs=2) as tb0, \
                     tc.tile_pool(name="tbxi", bufs=1) as tbx0, \
                     tc.tile_pool(name="tbpsi", bufs=2, space="PSUM") as tbps0:
                    nextk = 0
                    for g0 in stage_starts():
                        stage_group(p1s, 0, ufeat, cj_u, g0)
                        while nextk < UN and g0 + GT * P >= (nextk + 1) * SLAB:
                            build_unit(0, 0, nextk, tb0, tbx0, tbps0)
                            nextk += 1
                    while nextk < UN:
                        build_unit(0, 0, nextk, tb0, tbx0, tbps0)
                        nextk += 1
                nc.sync.dma_start(out=tbl[:, :, 0], in_=tblst.ap()[:])
                stage_side(1, ifeat, cj_m)
                for d in range(2):
                    yacc = yp.tile([P, TP, MSG], BF16, tag="yacc")
                    nc.vector.memset(yacc[:], 0.0)
                    for g in range(2):
                        ph = 2 * d + g
                        bstack = ExitStack()
                        if ph < 3:
                            btb = bstack.enter_context(
                                tc.tile_pool(name=f"tb{ph}", bufs=2))
                            btbx = bstack.enter_context(
                                tc.tile_pool(name=f"tbx{ph}", bufs=1))
                            btbps = bstack.enter_context(
                                tc.tile_pool(name=f"tbps{ph}", bufs=1,
                                             space="PSUM"))
                        bu = 0
                        # ---- gather + segment-sum for (d, g) ----
                        chunksl = chunks_all[d * 2 + g]
                        rows = rowsdg[d * 2 + g]
                        with tc.tile_pool(name="io", bufs=2) as iop, \
                             tc.tile_pool(name="go", bufs=2) as gop, \
                             tc.tile_pool(name="gb", bufs=2) as gbp, \
                             tc.tile_pool(name="oh", bufs=2) as ohp, \
                             tc.tile_pool(name="ts", bufs=2) as tsp, \
                             tc.tile_pool(name="mps", bufs=3,
                                          space="PSUM") as mps:
                            pos = 0
                            psy = None
                            for a0 in range(0, rows, NI):
                                ni = min(NI, rows - a0)
                                gi = iop.tile([P, NI // 16], I16, tag="gi")
                                nc.scalar.dma_start(
                                    out=gi[:, :ni // 16],
                                    in_=gidx.ap()[:, g16ofs + a0 // 16:
                                                  g16ofs + (a0 + ni) // 16])
                                dlt = iop.tile([P, NI // P, UN, 1], BF16,
                                               tag="dlt")
                                nc.scalar.dma_start(
                                    out=dlt[:, :ni // P, :, :],
                                    in_=dl8.ap()[:, ccofs + (a0 // P) * UN:
                                                 ccofs + ((a0 + ni) // P) * UN]
                                        .rearrange("p (c k one) -> p c k one",
                                                   k=UN, one=1))
                                go = gop.tile([P, NI, 1], F32, tag="go")
                                nc.gpsimd.ap_gather(
                                    go[:, :ni, :], tbl[:], gi[:, :ni // 16],
                                    channels=P, num_elems=NELEM, d=1,
                                    num_idxs=ni)
                                gb = gbp.tile([P, NI], BF16, tag="gbc")
                                nc.vector.tensor_copy(out=gb[:, :ni],
                                                      in_=go[:, :ni, 0])
                                for local in range(ni // P):
                                    t, r, first, last = chunksl[pos]
                                    pos += 1
                                    tps = mps.tile([P, P], BF16, space="PSUM",
                                                   tag="tps")
                                    nc.tensor.transpose(
                                        out=tps[:],
                                        in_=gb[:, local * P:(local + 1) * P],
                                        identity=ident16[:])
                                    tsb = tsp.tile([P, P], BF16, tag="tsb")
                                    nc.vector.tensor_copy(out=tsb[:], in_=tps[:])
                                    oh = ohp.tile([P, UN, P], BF16, tag="oh")
                                    nc.vector.tensor_tensor(
                                        out=oh[:],
                                        in0=dlt[:, local, :, :].to_broadcast(
                                            (P, UN, P)),
                                        in1=iota[:, 0:1, :].to_broadcast(
                                            (P, UN, P)),
                                        op=ALU.is_equal)
                                    if first:
                                        psy = mps.tile([P, 16], F32,
                                                       space="PSUM", tag="psy")
                                    for k in range(UN):
                                        nc.tensor.matmul(
                                            out=psy[:],
                                            lhsT=oh[:, k, :],
                                            rhs=tsb[:, 16 * k:16 * k + 16],
                                            start=(first and k == 0),
                                            stop=(last and k == UN - 1))
                                    if last:
                                        ys = yacc[:, t, r * 16:(r + 1) * 16]
                                        nc.vector.tensor_tensor(
                                            out=ys, in0=ys, in1=psy[:],
                                            op=ALU.add)
                                if ph < 3 and a0 > 0 and bu < UN:
                                    build_unit((ph + 1) // 2, (ph + 1) % 2,
                                               bu, btb, btbx, btbps)
                                    bu += 1
                        while ph < 3 and bu < UN:
                            build_unit((ph + 1) // 2, (ph + 1) % 2,
                                       bu, btb, btbx, btbps)
                            bu += 1
                        if ph < 3:
                            # load next phase's table now, ahead of the
                            # transform DMAs in the sync-engine FIFO
                            nc.sync.dma_start(out=tbl[:, :, 0],
                                              in_=tblst.ap()[:])
                        bstack.close()
                        g16ofs += rows // 16
                        ccofs += (rows // P) * UN
                    # ---------------- transform ----------------
                    with tc.tile_pool(name="p3", bufs=3) as p3, \
                         tc.tile_pool(name="p3ps", bufs=2, space="PSUM") as p3p:
                        for t in range(TP):
                            msg = p3.tile([P, MSG], F32, tag="msg")
                            nc.scalar.activation(
                                out=msg[:], in_=yacc[:, t, :],
                                func=ACTF.Relu,
                                scale=cisb[:, d * TP + t: d * TP + t + 1])
                            psmT = p3p.tile([MSG, P], F32, space="PSUM",
                                            tag="psmT")
                            nc.tensor.transpose(out=psmT[:], in_=msg[:],
                                                identity=ident[:])
                            msgT = p3.tile([MSG, P], F32, tag="msgT")
                            nc.vector.tensor_copy(out=msgT[:], in_=psmT[:])
                            fcp = p3p.tile([P, 64], F32, space="PSUM",
                                           tag="fcp")
                            nc.tensor.matmul(
                                out=fcp[:], lhsT=msgT[:], rhs=fcwT[:],
                                start=True, stop=True)
                            osb = p3.tile([P, 64], F32, tag="osb")
                            nc.vector.tensor_tensor(out=osb[:], in0=fcp[:],
                                                    in1=fcb[:], op=ALU.add)
                            dst = m_out if d == 0 else u_out
                            nc.sync.dma_start(
                                out=dst.ap()[t * P:(t + 1) * P], in_=osb[:])
    nc.compile()
    return nc


# ----------------------------------------------------------------- kernel

def make_in_maps(cfg, gidx, dl8, inputs):
    import ml_dtypes
    ins = {k: np.asarray(v) for k, v in inputs.items()}
    iota = np.tile(np.arange(P, dtype=ml_dtypes.bfloat16), (P, 1))
    dl8 = [d.astype(ml_dtypes.bfloat16) for d in dl8]
    base = dict(
        ufeat=ins['ufeat'], ifeat=ins['ifeat'],
        cj_u=ins['cj_user'], cj_m=ins['cj_movie'],
        attT=np.ascontiguousarray(ins['att'].T),
        basis2=ins['basis'].reshape(cfg.BAS, cfg.IN * 16).copy(),
        fc_w=ins['fc_w'], fc_b=ins['fc_b'].reshape(1, 64).copy(),
        iota128=iota,
    )
    in_maps = []
    for c in range(cfg.NCORES):
        ci = np.zeros((2 * cfg.TP * P, 1), np.float32)
        ci[:cfg.NSH] = ins['ci_movie'][c * cfg.NSH:(c + 1) * cfg.NSH]
        ci[cfg.TP * P:cfg.TP * P + cfg.NSH] = \
            ins['ci_user'][c * cfg.NSH:(c + 1) * cfg.NSH]
        in_maps.append({**base, 'ci_sh': ci, 'gidx': gidx[c], 'dl8': dl8[c]})
    return in_maps


def assemble(cfg, results):
    u = np.concatenate([results[c]['u_out'][:cfg.NSH]
                        for c in range(cfg.NCORES)])
    m = np.concatenate([results[c]['m_out'][:cfg.NSH]
                        for c in range(cfg.NCORES)])
    return u, m


def kernel(**inputs):
    from concourse import bass_utils
    cfg = Cfg()
    chunks_all, rowsdg, gidx, dl8 = build_plan(cfg, inputs['edge_user'],
                                               inputs['edge_movie'])
    nc = build_program(cfg, chunks_all, rowsdg, gidx[0].shape[1],
                       dl8[0].shape[1], cfg.NCORES)
    in_maps = make_in_maps(cfg, gidx, dl8, inputs)
    res = bass_utils.run_bass_kernel_spmd(nc, in_maps,
                                          core_ids=list(range(cfg.NCORES)))
    return assemble(cfg, res.results)
